# revision 8
# baseline (speedup 1.0000x reference)
"""Multi-head attention (RMSNorm-QK + RoPE + softmax + proj) on 8 Trainium2 cores.

v2 design (cost-model-driven rewrite of the baseline):
 - bf16 operands everywhere (matmuls cost 1 cyc/row like fp32r, but DVE gets
   2x modes and DMA halves); fp32 PSUM accumulation throughout.
 - Transposed PV: O tiles are [128 q, 65] (64 dims + ones col for the softmax
   denominator), using all 128 output partitions -> PV drops from 32768 to
   16640 cyc/head, the denominator becomes a per-partition column (reciprocal
   + tensor_scalar mult), and the old broadcast-reciprocal matmuls vanish.
 - O^T for the projection comes from PE transposes (128 bf16 rows each).
 - RMS rsqrt via exp(-0.5 ln x) on ACT (same table as the softmax exp; the
   DVE has no pow/divide/rsqrt ISA), qkv bias added in the DVE pipeline
   (per-partition scalar), v bias folded into the host-side proj bias
   (softmax rows sum to 1), softmax denominators via batched DVE reciprocal.
 - RoPE elementwise work split DVE/Pool; emission order software-pipelines
   S(k+1) ahead of exp(k), stages a phase's px tiles in SBUF so each O
   qb-region accumulates contiguously (PSUM start bit stays per-element
   correct on HW), defers phase closes ~1.25 phases so early PE work (qkv+v)
   overlaps the ACT-bound exp stream, and pumps qkv/v/proj filler chunks into
   the PE gaps.

Sharding: core c handles batch c//4 and heads [3*(c%4), 3*(c%4)+3).
Each core writes a bf16 [N, C] partial; the host sums 4 partials per batch
and adds proj_bias + qkv_bias[v-part] @ proj_kernel.
"""
import sys

for _p in ("/opt/trn_rl_repo", "/opt/trn_rl_repo/concourse"):
    if _p not in sys.path:
        sys.path.insert(0, _p)

from collections import deque
from contextlib import ExitStack

import ml_dtypes
import numpy as np

import concourse.bass as bass
import concourse.mybir as mybir
import concourse.tile as tile
from concourse.bass_utils import run_bass_kernel_spmd

F32 = mybir.dt.float32
BF16 = mybir.dt.bfloat16
AF = mybir.ActivationFunctionType
ALU = mybir.AluOpType
BF = ml_dtypes.bfloat16

B, N, C = 2, 2048, 768
H, HD = 12, 64
HP = 3            # heads per core
NCORES = 8
CCH = 6           # contraction chunks of 128
NT = 4            # token tiles of 512
KB = 16           # k blocks of 128
NG = 8            # 2-kb groups per (head, qtile) phase

SWAP_MASK = [(i + 16) % 32 for i in range(32)]
PERM = np.concatenate([np.arange(0, 16), np.arange(32, 48),
                       np.arange(16, 32), np.arange(48, 64)])
SIGN = np.where(PERM < 32, -1.0, 1.0).astype(np.float32)
# rope partner of PERM-position p (SWAP_MASK's intra-32 half swap)
SWAPIDX = np.array([(p // 32) * 32 + (p + 16) % 32 for p in range(64)])

_NC_CACHE = {}


def build_nc(split_waits=True):
    nc = bass.Bass(target_bir_lowering=True)
    xT = nc.declare_dram_parameter("xT", [C, N], BF16, isOutput=False)
    wqk = nc.declare_dram_parameter("wqk", [C, HP * 128], BF16, isOutput=False)
    cosw = nc.declare_dram_parameter("cosw", [128, N], BF16, isOutput=False)
    sinw = nc.declare_dram_parameter("sinw", [128, N], BF16, isOutput=False)
    wvp = nc.declare_dram_parameter("wvp", [128, CCH * HP * 64], BF16,
                                    isOutput=False)
    wp01 = nc.declare_dram_parameter("wp01", [128, C], BF16, isOutput=False)
    wp2 = nc.declare_dram_parameter("wp2", [64, C], BF16, isOutput=False)
    # consts: [onesp(2) | sel4(512) | ident(128)]
    consts = nc.declare_dram_parameter("consts", [128, 642], BF16,
                                       isOutput=False)
    bqk = nc.declare_dram_parameter("bqk", [128, HP], F32, isOutput=False)
    out = nc.declare_dram_parameter("out", [N, C], BF16, isOutput=True)

    with tile.TileContext(nc) as tc, ExitStack() as ctx:
        sb = ctx.enter_context(tc.tile_pool(name="sb", bufs=1))
        pipe = ctx.enter_context(tc.tile_pool(name="pipe", bufs=2))
        pxp = pipe     # per-tag bufs below
        otp = pipe
        pop = pipe
        # PSUM: 4 + 2 + 1 + 1 = 8 banks, one pool with per-tag bufs
        sp = ctx.enter_context(tc.tile_pool(name="sp", bufs=2, space="PSUM"))
        qp = sp
        op = sp
        mp = sp

        # ---------- static SBUF tiles ----------
        xs = sb.tile([128, CCH, N], BF16, tag="xs")
        wqk_sb = sb.tile([128, CCH, HP * 128], BF16, tag="wqk")
        wv_sb = sb.tile([128, CCH, HP * 64], BF16, tag="wv")
        cos_sb = sb.tile([128, N], BF16, tag="cos")
        sin_sb = sb.tile([128, N], BF16, tag="sin")
        cn = sb.tile([128, 642], BF16, tag="cn")
        onesp_sb = cn[:, 0:2]
        sel_sb = cn[:, 2:514]
        ident_sb = cn[:, 514:642]
        bqk_sb = sb.tile([128, HP], F32, tag="bqk")
        wp01_sb = sb.tile([128, C], BF16, tag="wp01")
        wp2_sb = sb.tile([64, C], BF16, tag="wp2")

        q12 = sb.tile([128, N], BF16, tag="q12")
        k12 = sb.tile([128, N], BF16, tag="k12")
        q3 = sb.tile([64, N], BF16, tag="q3")
        k3 = sb.tile([64, N], BF16, tag="k3")
        t4_all = sb.tile([128, N], BF16, tag="t4_all")
        s_sb = sb.tile([128, 512], F32, tag="s_sb")
        sv = sb.tile([128, 512], BF16, tag="sv")
        v3 = sb.tile([128, KB, HP, 65], BF16, tag="v3")
        ones48 = sb.tile([128, KB * HP], BF16, tag="ones48")
        o2 = sb.tile([128, NT, 4, 128], BF16, tag="o2")
        o1 = sb.tile([128, NT, 4, 64], BF16, tag="o1")

        def qT(h):
            return (q12[0:64], q12[64:128], q3[:])[h]

        def kT(h):
            return (k12[0:64], k12[64:128], k3[:])[h]

        # ---------- prologue DMAs (ordered for earliest qkv start) ----------
        xT_r = xT[:].rearrange("(c p) n -> p c n", p=128)
        wqk_r = wqk[:].rearrange("(c p) m -> p c m", p=128)
        d = nc.sync.dma_start
        d(wqk_sb[:, 0:2, :], wqk_r[:, 0:2, :])
        d(xs[:, 0:3, 0:512], xT_r[:, 0:3, 0:512])      # tile-0 tokens
        d(wqk_sb[:, 2:6, :], wqk_r[:, 2:6, :])
        d(xs[:, 3:6, 0:512], xT_r[:, 3:6, 0:512])
        d(bqk_sb[:], bqk[:, :])
        d(cn[:], consts[:, :])
        d(cos_sb[:, 0:1024], cosw[:, 0:1024])
        d(sin_sb[:, 0:1024], sinw[:, 0:1024])
        d(xs[:, :, 512:1024], xT_r[:, :, 512:1024])
        d(xs[:, :, 1024:1536], xT_r[:, :, 1024:1536])
        d(wv_sb[:].rearrange("p c m -> p (c m)"), wvp[:, :])
        d(xs[:, :, 1536:2048], xT_r[:, :, 1536:2048])
        d(cos_sb[:, 1024:2048], cosw[:, 1024:2048])
        d(sin_sb[:, 1024:2048], sinw[:, 1024:2048])
        d(wp01_sb[:], wp01[:, :])
        d(wp2_sb[:], wp2[:, :])

        nc.vector.memset(sv[:], 1.0)   # rows never written stay 1 (sel zeros them)
        nc.vector.memset(s_sb[:], 1.0)
        nc.vector.memset(ones48[:], 1.0)
        nc.vector.tensor_copy(
            v3[:].rearrange("p a b n -> p (a b) n", n=65)[:, :, 64], ones48[:])

        def mm(out_ap, lhsT, rhs, start, stop):
            nc.tensor.matmul(out_ap, lhsT, rhs, start=start, stop=stop,
                             skip_group_check=True)

        # ---------- qkv-head generator ----------
        # Per-tile chunks: mms -> RoPE pipe -> finA (sumsq+rsqrt) ->
        # finB (broadcast+scale). Emission defers fins so PE stays dense;
        # the qp ring (2) tolerates exactly one deferred finA.
        def qkv_gen(h):
            hs = slice(h * 128, (h + 1) * 128)
            qk = [None] * NT

            def mms(t):
                ts = slice(t * 512, (t + 1) * 512)
                qk[t] = qp.tile([128, 512], F32, tag="q", name=f"qk{t}")
                for c in range(CCH):
                    mm(qk[t][:], wqk_sb[:, c, hs], xs[:, c, ts], c == 0,
                       c == CCH - 1)

            def rope(t):
                ts = slice(t * 512, (t + 1) * 512)
                qkb = pipe.tile([128, 512], BF16, tag="qkb")
                nc.vector.tensor_scalar(qkb[:], qk[t][:], bqk_sb[:, h:h + 1],
                                        None, ALU.add)
                sq = pipe.tile([128, 512], BF16, tag="sq")
                if h == 0:
                    nc.vector.tensor_mul(sq[:], qkb[:], qkb[:])
                t1 = pipe.tile([128, 512], BF16, tag="t1")
                nc.gpsimd.tensor_mul(t1[:], qkb[:], cos_sb[:, ts])
                t2 = pipe.tile([128, 512], BF16, tag="t2")
                nc.vector.stream_shuffle(t2[:], qkb[:], SWAP_MASK)
                t3 = pipe.tile([128, 512], BF16, tag="t3")
                nc.vector.tensor_mul(t3[:], t2[:], sin_sb[:, ts])
                nc.vector.tensor_add(t4_all[:, ts], t1[:], t3[:])
                if h != 0:
                    nc.gpsimd.tensor_mul(sq[:], qkb[:], qkb[:])
                return sq

            def finA(t, sq):
                rows = slice(32 * t, 32 * t + 2)
                mm(qk[t][0:2, :], onesp_sb[:], sq[:], True, True)
                if h == 0:
                    # rsqrt = exp(-0.5 ln(ms)); same ACT table as softmax exp
                    lv = pipe.tile([2, 512], F32, tag="lv", name="lv")
                    nc.scalar.activation(lv[:], qk[t][0:2, :], AF.Ln,
                                         bias=0.0, scale=1.0 / HD)
                    nc.scalar.activation(sv[rows, :], lv[:], AF.Exp,
                                         bias=0.0, scale=-0.5)
                else:
                    nc.vector.tensor_copy(s_sb[rows, :], qk[t][0:2, :])

            def lnexp():
                lva = pipe.tile([128, 512], F32, tag="lva", name="lva")
                nc.scalar.activation(lva[:], s_sb[:], AF.Ln,
                                     bias=0.0, scale=1.0 / HD)
                nc.scalar.activation(sv[:], lva[:], AF.Exp, bias=0.0,
                                     scale=-0.5)

            def finB(t):
                ts = slice(t * 512, (t + 1) * 512)
                sqk_ps = qp.tile([128, 512], F32, tag="q")
                mm(sqk_ps[:], sel_sb[:, t * 128:(t + 1) * 128], sv[:],
                   True, True)
                nc.vector.tensor_mul(qT(h)[:, ts], t4_all[0:64, ts],
                                     sqk_ps[0:64, :])
                nc.vector.tensor_mul(kT(h)[:, ts], t4_all[64:128, ts],
                                     sqk_ps[64:128, :])

            sqs = [None] * NT

            def do_mms(t):
                mms(t)
                sqs[t] = rope(t)

            do_mms(0)
            yield 4500
            do_mms(1)
            yield 4500
            finA(0, sqs[0])
            yield 700
            do_mms(2)
            yield 4500
            finA(1, sqs[1])
            if h == 0:
                finB(0)
            yield 1600
            finA(2, sqs[2])
            yield 700
            do_mms(3)
            yield 4500
            if h == 0:
                finB(1)
            yield 900
            finA(3, sqs[3])
            yield 700
            if h != 0:
                lnexp()
                yield 700
                finB(0)
                yield 900
                finB(1)
                yield 900
            finB(2)
            yield 900
            finB(3)
            yield 900

        # ---------- v generator ----------
        def v_gen():
            for tt in range(KB):
                v_ps = qp.tile([128, HP * 64], F32, tag="q")
                for c in range(CCH):
                    mm(v_ps[:], xs[:, c, tt * 128:(tt + 1) * 128],
                       wv_sb[:, c, :], c == 0, c == CCH - 1)
                nc.vector.tensor_copy(
                    v3[:, tt, :, 0:64],
                    v_ps[:, :].rearrange("p (h n) -> p h n", h=HP))
                yield 1500

        # ---------- proj of one (qtile, qblock) ----------
        mtr = [None]

        def proj_qb(qt, qb):
            if mtr[0] is None:
                mtr[0] = mp.tile([128, 4, 128], F32, tag="m", name="mtr", bufs=1)
            m = mtr[0]
            tr01 = m[:, qb, 0:64].bitcast(BF16)
            tr2 = m[0:64, qb, 64:128].bitcast(BF16)
            nc.tensor.transpose(tr01, o2[:, qt, qb, :], ident_sb[:])
            nc.tensor.transpose(tr2, o1[:, qt, qb, :], ident_sb[:])
            on_act = qt == 3   # ACT is idle once the last exps drain
            ot01 = otp.tile([128, 128], BF16, tag="ot01", bufs=6)
            ot2 = otp.tile([64, 128], BF16, tag="ot2", bufs=6)
            if on_act:
                nc.scalar.activation(ot01[:], tr01, AF.Copy, bias=0.0,
                                     scale=1.0)
                nc.vector.tensor_copy(ot2[:], tr2)
            else:
                nc.vector.tensor_copy(ot01[:], tr01)
                nc.vector.tensor_copy(ot2[:], tr2)
            po = pop.tile([128, C], BF16, tag="po", bufs=3)
            for half in range(2):
                cs = slice(half * 384, (half + 1) * 384)
                p_ps = qp.tile([128, 384], F32, tag="q")
                mm(p_ps[:], ot01[:], wp01_sb[:, cs], True, False)
                mm(p_ps[:], ot2[:], wp2_sb[:, cs], False, True)
                if on_act and half == 1:
                    nc.scalar.activation(po[:, cs], p_ps[:], AF.Copy,
                                         bias=0.0, scale=1.0)
                else:
                    nc.vector.tensor_copy(po[:, cs], p_ps[:])
            tb = qt * 4 + qb
            nc.sync.dma_start(out[tb * 128:(tb + 1) * 128, :], po[:])

        # ---------- filler pump ----------
        fillers = deque()
        debt = [0.0]

        def pump(budget):
            budget += debt[0]
            while budget > 0 and fillers:
                try:
                    budget -= next(fillers[0])
                except StopIteration:
                    fillers.popleft()
            debt[0] = min(budget, 3000.0)

        def ensure_done(gen):
            """Pump until `gen` has fully emitted (emission-order guard for
            cross-generator data deps)."""
            while gen in fillers:
                pump(100000)

        # ---------- attention stream ----------
        phases = [(h, qt) for h in range(HP) for qt in range(NT)]
        px_tiles = {}
        emitted = set()

        def emit_group(p, g):
            if (p, g) in emitted:
                return
            emitted.add((p, g))
            h, qt = phases[p]
            qs = slice(qt * 512, (qt + 1) * 512)
            s_ps = sp.tile([128, 1024], F32, tag="s")
            for j in range(2):
                kb = 2 * g + j
                mm(s_ps[:, j * 512:(j + 1) * 512],
                   kT(h)[:, kb * 128:(kb + 1) * 128], qT(h)[:, qs],
                   True, True)
            px = pxp.tile([128, 1024], BF16, tag="px", bufs=28)
            nc.scalar.activation(px[:], s_ps[:], AF.Exp, bias=0.0, scale=0.125)
            px_tiles[(p, g)] = px

        def close_gen(p):
            if p == 0:
                ensure_done(vg)   # PV reads v3; emission-order guard
            h, qt = phases[p]
            o_ps = op.tile([128, 4, 65], F32, tag="o", bufs=1)
            for qb in range(4):
                for g in range(NG):
                    px = px_tiles[(p, g)]
                    for j in range(2):
                        kb = 2 * g + j
                        mm(o_ps[:, qb, :],
                           px[:, j * 512 + qb * 128:j * 512 + (qb + 1) * 128],
                           v3[:, kb, h, :],
                           qb == 0 and kb == 0, kb == KB - 1)
                if qb == 1 or qb == 3:
                    yield
            # normalize by the ones-column denominators (batched reciprocal,
            # then per-qb per-partition multiply); epilogues after ALL PV so
            # coarse WAR tracking can't serialize the qb bundles
            rec4 = pipe.tile([128, 4], F32, tag="rec4", name="rec4")
            nc.vector.reciprocal(rec4[:], o_ps[:, :, 64])
            for qb in range(4):
                dst = (o2[:, qt, qb, h * 64:(h + 1) * 64] if h < 2
                       else o1[:, qt, qb, :])
                nc.vector.tensor_scalar(dst, o_ps[:, qb, 0:64],
                                        rec4[:, qb:qb + 1], None, ALU.mult)
                if h == 2:
                    proj_qb(qt, qb)
                yield
            for g in range(NG):
                del px_tiles[(p, g)]

        def drain_close(cg):
            for _ in cg:
                pass

        # ---------- main schedule ----------
        # Phase 0 runs with qkv(h0) inlined per tile: tile t unlocks S groups
        # 2t, 2t+1 (k-tiles) while qT(qt0) comes entirely from tile 0.
        qg0 = qkv_gen(0)
        vg = v_gen()
        g1, g2 = qkv_gen(1), qkv_gen(2)
        fillers.append(vg)
        fillers.append(g1)
        need_gen = {1: g1, 2: g2}

        def drain_n(gen, n_chunks):
            for _ in range(n_chunks):
                next(gen)

        # Front: dense qkv-h0 mms with the exp stream fed by EVERY group
        # whose gates are open. Tile t gates k-blocks 4t..4t+3 (groups
        # 2t,2t+1 of every h0 phase) and the q-tokens of phase (0,t).
        drain_n(qg0, 5)              # thru finB(0)
        front = [(0, 0), (0, 1),
                 "T1", (0, 2), (0, 3), (1, 0), (1, 1),
                 "T2", (0, 4), (0, 5), (1, 2), (1, 3), (2, 0), (2, 1),
                 "T3", (0, 6), (0, 7), (1, 4), (1, 5), (2, 2), (2, 3),
                 (3, 0), (3, 1)]
        for item in front:
            if item == "T1":
                drain_n(qg0, 3)      # finA2, mms3, finB1
            elif item == "T2":
                drain_n(qg0, 2)      # finA3, finB2
            elif item == "T3":
                drain_n(qg0, 1)      # finB3
            else:
                emit_group(*item)
                pump(600)

        # Steady state: early closes deferred ~2 phases (px ring holds ~3
        # phases) so v/qkv fillers use the early PE slack; late closes pulled
        # in so proj work overlaps the remaining exp stream.
        NP = len(phases)
        close_at = {}
        for p in range(NP - 1):
            # early closes deferred 3 phases (shifts PV out of the PE-heavy
            # qkv/v window); h2 closes pulled in so proj overlaps exps
            lag = (p + 3, 1) if p < 8 else (p + 1, 0)
            close_at.setdefault(lag, []).append(p)

        def after_close(cp):
            if phases[cp] == (0, 2):
                fillers.append(g2)

        active_closes = []
        for p in range(1, NP):
            nh = phases[p][0]
            if nh != phases[p - 1][0]:
                ensure_done(need_gen[nh])
            for g in range(NG):
                emit_group(p, g)
                pump(2400 if p <= 4 else 2200)
                for cp in close_at.get((p, g), []):
                    active_closes.append((cp, close_gen(cp)))
                if active_closes:
                    steps = 2 if p >= NP - 2 else 1
                    for _ in range(steps):
                        if not active_closes:
                            break
                        cp, cg = active_closes[0]
                        try:
                            next(cg)
                        except StopIteration:
                            active_closes.pop(0)
                            after_close(cp)
        for cp, cg in active_closes:
            drain_close(cg)
            after_close(cp)
        drain_close(close_gen(NP - 1))
        while fillers:
            pump(100000)

    if split_waits:
        _split_waits(nc)
    return nc


def _split_waits(nc):
    """Walrus lowers at most one sync-wait per instruction; move excess waits
    onto NoOps inserted just before, on the same engine queue."""
    k = 0
    for fn in nc.m.functions:
        for bb in fn.blocks:
            il = bb.instructions
            idx = 0
            while idx < len(il):
                inst = il[idx]
                si = inst.sync_info
                eng = getattr(inst, "engine", None)
                if (si is not None and len(si.on_wait) > 1
                        and eng is not None
                        and str(eng) != "EngineType.Unassigned"):
                    waits = list(si.on_wait)
                    inst.sync_info = mybir.SyncInfo(
                        on_wait=[waits[-1]], on_update=list(si.on_update))
                    for w in waits[:-1]:
                        nop = mybir.InstNoOp(
                            name=f"I-waitnop-{k}", engine=eng, ins=[], outs=[],
                            sync_info=mybir.SyncInfo(on_wait=[w], on_update=[]))
                        k += 1
                        il.insert(idx, nop)
                        idx += 1
                idx += 1


def _prep_core_inputs(core, x, rope_cos, rope_sin, qkv_kernel, qkv_bias,
                      proj_kernel, proj_bias, q_norm_w, k_norm_w):
    b = core // 4
    heads = [3 * (core % 4) + i for i in range(HP)]

    wq = qkv_kernel.reshape(C, 3, H, HD)
    bq = qkv_bias.reshape(3, H, HD)

    xTa = np.ascontiguousarray(x[b].T).astype(BF)

    wqk = np.empty((C, HP * 128), np.float32)
    bqk = np.zeros((128, HP), np.float32)
    for i, h in enumerate(heads):
        wqk[:, i * 128:i * 128 + 64] = wq[:, 0, h, PERM]
        wqk[:, i * 128 + 64:(i + 1) * 128] = wq[:, 1, h, PERM]
        bqk[0:64, i] = bq[0, h, PERM]
        bqk[64:128, i] = bq[1, h, PERM]

    wv = np.zeros((C, HP * 64), np.float32)
    for i, h in enumerate(heads):
        wv[:, i * 64:(i + 1) * 64] = wq[:, 2, h, :]
    # packed [p, (c m)] so the SBUF copy is one dense DMA
    wvp = wv.reshape(CCH, 128, HP * 64).transpose(1, 0, 2).reshape(128, -1)

    cosT = rope_cos.T  # (HD, N)
    sinT = rope_sin.T
    cosw = np.empty((128, N), np.float32)
    sinw = np.empty((128, N), np.float32)
    cosw[0:64] = cosT[PERM] * q_norm_w[PERM][:, None]
    cosw[64:128] = cosT[PERM] * k_norm_w[PERM][:, None]
    # sin multiplies the SHUFFLED (partner) value -> partner's norm weight
    qn_p = q_norm_w[PERM][SWAPIDX]
    kn_p = k_norm_w[PERM][SWAPIDX]
    sinw[0:64] = SIGN[:, None] * sinT[PERM] * qn_p[:, None]
    sinw[64:128] = SIGN[:, None] * sinT[PERM] * kn_p[:, None]

    onesp = np.zeros((128, 2), np.float32)
    onesp[0:64, 0] = 1.0
    onesp[64:128, 1] = 1.0

    sel4 = np.zeros((128, 512), np.float32)
    for t in range(NT):
        sel4[32 * t, t * 128:t * 128 + 64] = 1.0
        sel4[32 * t + 1, t * 128 + 64:(t + 1) * 128] = 1.0

    rows01 = np.concatenate([np.arange(h * HD, (h + 1) * HD)
                             for h in heads[0:2]])
    rows2 = np.arange(heads[2] * HD, (heads[2] + 1) * HD)
    wp01 = proj_kernel[rows01, :]
    wp2 = proj_kernel[rows2, :]

    consts = np.zeros((128, 642), np.float32)
    consts[:, 0:2] = onesp
    consts[:, 2:514] = sel4
    consts[:, 514:642] = np.eye(128, dtype=np.float32)
    return {"xT": xTa, "wqk": wqk.astype(BF), "bqk": bqk,
            "cosw": cosw.astype(BF), "sinw": sinw.astype(BF),
            "wvp": np.ascontiguousarray(wvp).astype(BF),
            "wp01": np.ascontiguousarray(wp01).astype(BF),
            "wp2": np.ascontiguousarray(wp2).astype(BF),
            "consts": consts.astype(BF)}


def kernel(x, rope_cos, rope_sin, qkv_kernel, qkv_bias, proj_kernel,
           proj_bias, q_norm_w, k_norm_w, _trace=False):
    args = [np.asarray(a, dtype=np.float32) for a in
            (x, rope_cos, rope_sin, qkv_kernel, qkv_bias, proj_kernel,
             proj_bias, q_norm_w, k_norm_w)]
    in_maps = [_prep_core_inputs(c, *args) for c in range(NCORES)]

    if "nc" not in _NC_CACHE:
        _NC_CACHE["nc"] = build_nc()
    nc = _NC_CACHE["nc"]

    res = run_bass_kernel_spmd(nc, in_maps, core_ids=list(range(NCORES)),
                               trace=_trace)
    parts = [np.asarray(res.results[c]["out"]).astype(np.float32)
             for c in range(NCORES)]
    # v-bias contributes exactly bv @ proj_kernel (softmax rows sum to 1)
    pb = (np.asarray(proj_bias, dtype=np.float32)
          + np.asarray(qkv_bias, dtype=np.float32)[2 * C:]
          @ np.asarray(proj_kernel, dtype=np.float32))
    out = np.empty((B, N, C), np.float32)
    for b in range(B):
        out[b] = (parts[4 * b] + parts[4 * b + 1] + parts[4 * b + 2]
                  + parts[4 * b + 3] + pb)
    if _trace:
        kernel.last_results = res
    return out


# revision 9
# speedup vs baseline: 1.0002x; 1.0002x over previous
"""Multi-head attention (RMSNorm-QK + RoPE + softmax + proj) on 8 Trainium2 cores.

v2 design (cost-model-driven rewrite of the baseline):
 - bf16 operands everywhere (matmuls cost 1 cyc/row like fp32r, but DVE gets
   2x modes and DMA halves); fp32 PSUM accumulation throughout.
 - Transposed PV: O tiles are [128 q, 65] (64 dims + ones col for the softmax
   denominator), using all 128 output partitions -> PV drops from 32768 to
   16640 cyc/head, the denominator becomes a per-partition column (reciprocal
   + tensor_scalar mult), and the old broadcast-reciprocal matmuls vanish.
 - O^T for the projection comes from PE transposes (128 bf16 rows each).
 - RMS rsqrt via exp(-0.5 ln x) on ACT (same table as the softmax exp; the
   DVE has no pow/divide/rsqrt ISA), qkv bias added in the DVE pipeline
   (per-partition scalar), v bias folded into the host-side proj bias
   (softmax rows sum to 1), softmax denominators via batched DVE reciprocal.
 - RoPE elementwise work split DVE/Pool; emission order software-pipelines
   S(k+1) ahead of exp(k), stages a phase's px tiles in SBUF so each O
   qb-region accumulates contiguously (PSUM start bit stays per-element
   correct on HW), defers phase closes ~1.25 phases so early PE work (qkv+v)
   overlaps the ACT-bound exp stream, and pumps qkv/v/proj filler chunks into
   the PE gaps.

Sharding: core c handles batch c//4 and heads [3*(c%4), 3*(c%4)+3).
Each core writes a bf16 [N, C] partial; the host sums 4 partials per batch
and adds proj_bias + qkv_bias[v-part] @ proj_kernel.
"""
import sys

for _p in ("/opt/trn_rl_repo", "/opt/trn_rl_repo/concourse"):
    if _p not in sys.path:
        sys.path.insert(0, _p)

from collections import deque
from contextlib import ExitStack

import ml_dtypes
import numpy as np

import concourse.bass as bass
import concourse.mybir as mybir
import concourse.tile as tile
from concourse.bass_utils import run_bass_kernel_spmd

F32 = mybir.dt.float32
BF16 = mybir.dt.bfloat16
AF = mybir.ActivationFunctionType
ALU = mybir.AluOpType
BF = ml_dtypes.bfloat16

B, N, C = 2, 2048, 768
H, HD = 12, 64
HP = 3            # heads per core
NCORES = 8
CCH = 6           # contraction chunks of 128
NT = 4            # token tiles of 512
KB = 16           # k blocks of 128
NG = 8            # 2-kb groups per (head, qtile) phase

SWAP_MASK = [(i + 16) % 32 for i in range(32)]
PERM = np.concatenate([np.arange(0, 16), np.arange(32, 48),
                       np.arange(16, 32), np.arange(48, 64)])
SIGN = np.where(PERM < 32, -1.0, 1.0).astype(np.float32)
# rope partner of PERM-position p (SWAP_MASK's intra-32 half swap)
SWAPIDX = np.array([(p // 32) * 32 + (p + 16) % 32 for p in range(64)])

_NC_CACHE = {}


def build_nc(split_waits=True):
    nc = bass.Bass(target_bir_lowering=True)
    xT = nc.declare_dram_parameter("xT", [C, N], BF16, isOutput=False)
    wqk = nc.declare_dram_parameter("wqk", [C, HP * 128], BF16, isOutput=False)
    cosw = nc.declare_dram_parameter("cosw", [128, N], BF16, isOutput=False)
    sinw = nc.declare_dram_parameter("sinw", [128, N], BF16, isOutput=False)
    wvp = nc.declare_dram_parameter("wvp", [128, CCH * HP * 64], BF16,
                                    isOutput=False)
    wp01 = nc.declare_dram_parameter("wp01", [128, C], BF16, isOutput=False)
    wp2 = nc.declare_dram_parameter("wp2", [64, C], BF16, isOutput=False)
    # consts: [onesp(2) | sel4(512) | ident(128)]
    consts = nc.declare_dram_parameter("consts", [128, 642], BF16,
                                       isOutput=False)
    bqk = nc.declare_dram_parameter("bqk", [128, HP], F32, isOutput=False)
    out = nc.declare_dram_parameter("out", [N, C], BF16, isOutput=True)

    with tile.TileContext(nc) as tc, ExitStack() as ctx:
        sb = ctx.enter_context(tc.tile_pool(name="sb", bufs=1))
        pipe = ctx.enter_context(tc.tile_pool(name="pipe", bufs=2))
        pxp = pipe     # per-tag bufs below
        otp = pipe
        pop = pipe
        # PSUM: 4 + 2 + 1 + 1 = 8 banks, one pool with per-tag bufs
        sp = ctx.enter_context(tc.tile_pool(name="sp", bufs=2, space="PSUM"))
        qp = sp
        op = sp
        mp = sp

        # ---------- static SBUF tiles ----------
        xs = sb.tile([128, CCH, N], BF16, tag="xs")
        wqk_sb = sb.tile([128, CCH, HP * 128], BF16, tag="wqk")
        wv_sb = sb.tile([128, CCH, HP * 64], BF16, tag="wv")
        cos_sb = sb.tile([128, N], BF16, tag="cos")
        sin_sb = sb.tile([128, N], BF16, tag="sin")
        cn = sb.tile([128, 642], BF16, tag="cn")
        onesp_sb = cn[:, 0:2]
        sel_sb = cn[:, 2:514]
        ident_sb = cn[:, 514:642]
        bqk_sb = sb.tile([128, HP], F32, tag="bqk")
        wp01_sb = sb.tile([128, C], BF16, tag="wp01")
        wp2_sb = sb.tile([64, C], BF16, tag="wp2")

        q12 = sb.tile([128, N], BF16, tag="q12")
        k12 = sb.tile([128, N], BF16, tag="k12")
        q3 = sb.tile([64, N], BF16, tag="q3")
        k3 = sb.tile([64, N], BF16, tag="k3")
        t4_all = sb.tile([128, N], BF16, tag="t4_all")
        s_sb = sb.tile([128, 512], F32, tag="s_sb")
        sv = sb.tile([128, 512], BF16, tag="sv")
        v3 = sb.tile([128, KB, HP, 65], BF16, tag="v3")
        ones48 = sb.tile([128, KB * HP], BF16, tag="ones48")
        o2 = sb.tile([128, NT, 4, 128], BF16, tag="o2")
        o1 = sb.tile([128, NT, 4, 64], BF16, tag="o1")

        def qT(h):
            return (q12[0:64], q12[64:128], q3[:])[h]

        def kT(h):
            return (k12[0:64], k12[64:128], k3[:])[h]

        # ---------- prologue DMAs (ordered for earliest qkv start) ----------
        xT_r = xT[:].rearrange("(c p) n -> p c n", p=128)
        wqk_r = wqk[:].rearrange("(c p) m -> p c m", p=128)
        d = nc.sync.dma_start
        d(wqk_sb[:, 0:2, :], wqk_r[:, 0:2, :])
        d(xs[:, 0:3, 0:512], xT_r[:, 0:3, 0:512])      # tile-0 tokens
        d(wqk_sb[:, 2:6, :], wqk_r[:, 2:6, :])
        d(xs[:, 3:6, 0:512], xT_r[:, 3:6, 0:512])
        d(bqk_sb[:], bqk[:, :])
        d(cn[:], consts[:, :])
        d(cos_sb[:, 0:1024], cosw[:, 0:1024])
        d(sin_sb[:, 0:1024], sinw[:, 0:1024])
        d(xs[:, :, 512:1024], xT_r[:, :, 512:1024])
        d(xs[:, :, 1024:1536], xT_r[:, :, 1024:1536])
        d(wv_sb[:].rearrange("p c m -> p (c m)"), wvp[:, :])
        d(xs[:, :, 1536:2048], xT_r[:, :, 1536:2048])
        d(cos_sb[:, 1024:2048], cosw[:, 1024:2048])
        d(sin_sb[:, 1024:2048], sinw[:, 1024:2048])
        d(wp01_sb[:], wp01[:, :])
        d(wp2_sb[:], wp2[:, :])

        nc.vector.memset(sv[:], 1.0)   # rows never written stay 1 (sel zeros them)
        nc.vector.memset(s_sb[:], 1.0)
        nc.vector.memset(ones48[:], 1.0)
        nc.vector.tensor_copy(
            v3[:].rearrange("p a b n -> p (a b) n", n=65)[:, :, 64], ones48[:])

        def mm(out_ap, lhsT, rhs, start, stop):
            nc.tensor.matmul(out_ap, lhsT, rhs, start=start, stop=stop,
                             skip_group_check=True)

        # ---------- qkv-head generator ----------
        # Per-tile chunks: mms -> RoPE pipe -> finA (sumsq+rsqrt) ->
        # finB (broadcast+scale). Emission defers fins so PE stays dense;
        # the qp ring (2) tolerates exactly one deferred finA.
        def qkv_gen(h):
            hs = slice(h * 128, (h + 1) * 128)
            qk = [None] * NT

            def mms(t):
                ts = slice(t * 512, (t + 1) * 512)
                qk[t] = qp.tile([128, 512], F32, tag="q", name=f"qk{t}")
                for c in range(CCH):
                    mm(qk[t][:], wqk_sb[:, c, hs], xs[:, c, ts], c == 0,
                       c == CCH - 1)

            def rope(t):
                ts = slice(t * 512, (t + 1) * 512)
                qkb = pipe.tile([128, 512], BF16, tag="qkb")
                nc.vector.tensor_scalar(qkb[:], qk[t][:], bqk_sb[:, h:h + 1],
                                        None, ALU.add)
                sq = pipe.tile([128, 512], BF16, tag="sq")
                if h == 0:
                    nc.vector.tensor_mul(sq[:], qkb[:], qkb[:])
                t1 = pipe.tile([128, 512], BF16, tag="t1")
                nc.gpsimd.tensor_mul(t1[:], qkb[:], cos_sb[:, ts])
                t2 = pipe.tile([128, 512], BF16, tag="t2")
                nc.vector.stream_shuffle(t2[:], qkb[:], SWAP_MASK)
                t3 = pipe.tile([128, 512], BF16, tag="t3")
                nc.vector.tensor_mul(t3[:], t2[:], sin_sb[:, ts])
                nc.vector.tensor_add(t4_all[:, ts], t1[:], t3[:])
                if h != 0:
                    nc.gpsimd.tensor_mul(sq[:], qkb[:], qkb[:])
                return sq

            def finA(t, sq):
                rows = slice(32 * t, 32 * t + 2)
                mm(qk[t][0:2, :], onesp_sb[:], sq[:], True, True)
                if h == 0:
                    # rsqrt = exp(-0.5 ln(ms)); same ACT table as softmax exp
                    lv = pipe.tile([2, 512], F32, tag="lv", name="lv")
                    nc.scalar.activation(lv[:], qk[t][0:2, :], AF.Ln,
                                         bias=0.0, scale=1.0 / HD)
                    nc.scalar.activation(sv[rows, :], lv[:], AF.Exp,
                                         bias=0.0, scale=-0.5)
                else:
                    nc.vector.tensor_copy(s_sb[rows, :], qk[t][0:2, :])

            def lnexp():
                lva = pipe.tile([128, 512], F32, tag="lva", name="lva")
                nc.scalar.activation(lva[:], s_sb[:], AF.Ln,
                                     bias=0.0, scale=1.0 / HD)
                nc.scalar.activation(sv[:], lva[:], AF.Exp, bias=0.0,
                                     scale=-0.5)

            def finB(t):
                ts = slice(t * 512, (t + 1) * 512)
                sqk_ps = qp.tile([128, 512], F32, tag="q")
                mm(sqk_ps[:], sel_sb[:, t * 128:(t + 1) * 128], sv[:],
                   True, True)
                nc.vector.tensor_mul(qT(h)[:, ts], t4_all[0:64, ts],
                                     sqk_ps[0:64, :])
                nc.vector.tensor_mul(kT(h)[:, ts], t4_all[64:128, ts],
                                     sqk_ps[64:128, :])

            sqs = [None] * NT

            def do_mms(t):
                mms(t)
                sqs[t] = rope(t)

            do_mms(0)
            yield 4500
            do_mms(1)
            yield 4500
            finA(0, sqs[0])
            yield 700
            do_mms(2)
            yield 4500
            finA(1, sqs[1])
            if h == 0:
                finB(0)
            yield 1600
            finA(2, sqs[2])
            yield 700
            do_mms(3)
            yield 4500
            if h == 0:
                finB(1)
            yield 900
            finA(3, sqs[3])
            yield 700
            if h != 0:
                lnexp()
                yield 700
                finB(0)
                yield 900
                finB(1)
                yield 900
            finB(2)
            yield 900
            finB(3)
            yield 900

        # ---------- v generator ----------
        def v_gen():
            for tt in range(KB):
                v_ps = qp.tile([128, HP * 64], F32, tag="q")
                for c in range(CCH):
                    mm(v_ps[:], xs[:, c, tt * 128:(tt + 1) * 128],
                       wv_sb[:, c, :], c == 0, c == CCH - 1)
                nc.vector.tensor_copy(
                    v3[:, tt, :, 0:64],
                    v_ps[:, :].rearrange("p (h n) -> p h n", h=HP))
                yield 1500

        # ---------- proj of one (qtile, qblock) ----------
        mtr = [None]

        def proj_qb(qt, qb):
            if mtr[0] is None:
                mtr[0] = mp.tile([128, 4, 128], F32, tag="m", name="mtr", bufs=1)
            m = mtr[0]
            tr01 = m[:, qb, 0:64].bitcast(BF16)
            tr2 = m[0:64, qb, 64:128].bitcast(BF16)
            nc.tensor.transpose(tr01, o2[:, qt, qb, :], ident_sb[:])
            nc.tensor.transpose(tr2, o1[:, qt, qb, :], ident_sb[:])
            on_act = qt == 3   # ACT is idle once the last exps drain
            ot01 = otp.tile([128, 128], BF16, tag="ot01", bufs=6)
            ot2 = otp.tile([64, 128], BF16, tag="ot2", bufs=6)
            if on_act:
                nc.scalar.activation(ot01[:], tr01, AF.Copy, bias=0.0,
                                     scale=1.0)
                nc.vector.tensor_copy(ot2[:], tr2)
            else:
                nc.vector.tensor_copy(ot01[:], tr01)
                nc.vector.tensor_copy(ot2[:], tr2)
            po = pop.tile([128, C], BF16, tag="po", bufs=3)
            for half in range(2):
                cs = slice(half * 384, (half + 1) * 384)
                p_ps = qp.tile([128, 384], F32, tag="q")
                mm(p_ps[:], ot01[:], wp01_sb[:, cs], True, False)
                mm(p_ps[:], ot2[:], wp2_sb[:, cs], False, True)
                if on_act and half == 1:
                    nc.scalar.activation(po[:, cs], p_ps[:], AF.Copy,
                                         bias=0.0, scale=1.0)
                else:
                    nc.vector.tensor_copy(po[:, cs], p_ps[:])
            tb = qt * 4 + qb
            nc.sync.dma_start(out[tb * 128:(tb + 1) * 128, :], po[:])

        # ---------- filler pump ----------
        fillers = deque()
        debt = [0.0]

        def pump(budget):
            budget += debt[0]
            while budget > 0 and fillers:
                try:
                    budget -= next(fillers[0])
                except StopIteration:
                    fillers.popleft()
            debt[0] = min(budget, 3000.0)

        def ensure_done(gen):
            """Pump until `gen` has fully emitted (emission-order guard for
            cross-generator data deps)."""
            while gen in fillers:
                pump(100000)

        # ---------- attention stream ----------
        phases = [(h, qt) for h in range(HP) for qt in range(NT)]
        px_tiles = {}
        emitted = set()

        def emit_group(p, g):
            if (p, g) in emitted:
                return
            emitted.add((p, g))
            h, qt = phases[p]
            qs = slice(qt * 512, (qt + 1) * 512)
            s_ps = sp.tile([128, 1024], F32, tag="s")
            for j in range(2):
                kb = 2 * g + j
                mm(s_ps[:, j * 512:(j + 1) * 512],
                   kT(h)[:, kb * 128:(kb + 1) * 128], qT(h)[:, qs],
                   True, True)
            px = pxp.tile([128, 1024], BF16, tag="px", bufs=28)
            nc.scalar.activation(px[:], s_ps[:], AF.Exp, bias=0.0, scale=0.125)
            px_tiles[(p, g)] = px

        def close_gen(p):
            if p == 0:
                ensure_done(vg)   # PV reads v3; emission-order guard
            h, qt = phases[p]
            o_ps = op.tile([128, 4, 65], F32, tag="o", bufs=1)
            for qb in range(4):
                for g in range(NG):
                    px = px_tiles[(p, g)]
                    for j in range(2):
                        kb = 2 * g + j
                        mm(o_ps[:, qb, :],
                           px[:, j * 512 + qb * 128:j * 512 + (qb + 1) * 128],
                           v3[:, kb, h, :],
                           qb == 0 and kb == 0, kb == KB - 1)
                if qb == 1 or qb == 3:
                    yield
            # normalize by the ones-column denominators (batched reciprocal,
            # then per-qb per-partition multiply); epilogues after ALL PV so
            # coarse WAR tracking can't serialize the qb bundles
            rec4 = pipe.tile([128, 4], F32, tag="rec4", name="rec4")
            nc.vector.reciprocal(rec4[:], o_ps[:, :, 64])
            for qb in range(4):
                dst = (o2[:, qt, qb, h * 64:(h + 1) * 64] if h < 2
                       else o1[:, qt, qb, :])
                nc.vector.tensor_scalar(dst, o_ps[:, qb, 0:64],
                                        rec4[:, qb:qb + 1], None, ALU.mult)
                if h == 2:
                    proj_qb(qt, qb)
                yield
            for g in range(NG):
                del px_tiles[(p, g)]

        def drain_close(cg):
            for _ in cg:
                pass

        # ---------- main schedule ----------
        # Phase 0 runs with qkv(h0) inlined per tile: tile t unlocks S groups
        # 2t, 2t+1 (k-tiles) while qT(qt0) comes entirely from tile 0.
        qg0 = qkv_gen(0)
        vg = v_gen()
        g1, g2 = qkv_gen(1), qkv_gen(2)
        fillers.append(vg)
        fillers.append(g1)
        need_gen = {1: g1, 2: g2}

        def drain_n(gen, n_chunks):
            for _ in range(n_chunks):
                next(gen)

        # Front: dense qkv-h0 mms with the exp stream fed by EVERY group
        # whose gates are open. Tile t gates k-blocks 4t..4t+3 (groups
        # 2t,2t+1 of every h0 phase) and the q-tokens of phase (0,t).
        drain_n(qg0, 5)              # thru finB(0)
        front = [(0, 0), (0, 1),
                 "T1", (0, 2), (0, 3), (1, 0), (1, 1),
                 "T2", (0, 4), (0, 5), (1, 2), (1, 3), (2, 0), (2, 1),
                 "T3", (0, 6), (0, 7), (1, 4), (1, 5), (2, 2), (2, 3),
                 (3, 0), (3, 1)]
        for item in front:
            if item == "T1":
                drain_n(qg0, 3)      # finA2, mms3, finB1
            elif item == "T2":
                drain_n(qg0, 2)      # finA3, finB2
            elif item == "T3":
                drain_n(qg0, 1)      # finB3
            else:
                emit_group(*item)
                pump(600)

        # Steady state: early closes deferred ~2 phases (px ring holds ~3
        # phases) so v/qkv fillers use the early PE slack; late closes pulled
        # in so proj work overlaps the remaining exp stream.
        NP = len(phases)
        close_at = {}
        for p in range(NP - 1):
            # early closes deferred 3 phases (shifts PV out of the PE-heavy
            # qkv/v window); h2 closes pulled in so proj overlaps exps
            lag = (p + 3, 1) if p < 8 else (p + 1, 0)
            close_at.setdefault(lag, []).append(p)

        def after_close(cp):
            if phases[cp] == (0, 2):
                fillers.append(g2)

        active_closes = []
        for p in range(1, NP):
            nh = phases[p][0]
            if nh != phases[p - 1][0]:
                ensure_done(need_gen[nh])
            for g in range(NG):
                emit_group(p, g)
                pump(2400)
                for cp in close_at.get((p, g), []):
                    active_closes.append((cp, close_gen(cp)))
                if active_closes:
                    steps = 2 if p >= NP - 2 else 1
                    for _ in range(steps):
                        if not active_closes:
                            break
                        cp, cg = active_closes[0]
                        try:
                            next(cg)
                        except StopIteration:
                            active_closes.pop(0)
                            after_close(cp)
        for cp, cg in active_closes:
            drain_close(cg)
            after_close(cp)
        drain_close(close_gen(NP - 1))
        while fillers:
            pump(100000)

    if split_waits:
        _split_waits(nc)
    return nc


def _split_waits(nc):
    """Walrus lowers at most one sync-wait per instruction; move excess waits
    onto NoOps inserted just before, on the same engine queue."""
    k = 0
    for fn in nc.m.functions:
        for bb in fn.blocks:
            il = bb.instructions
            idx = 0
            while idx < len(il):
                inst = il[idx]
                si = inst.sync_info
                eng = getattr(inst, "engine", None)
                if (si is not None and len(si.on_wait) > 1
                        and eng is not None
                        and str(eng) != "EngineType.Unassigned"):
                    waits = list(si.on_wait)
                    inst.sync_info = mybir.SyncInfo(
                        on_wait=[waits[-1]], on_update=list(si.on_update))
                    for w in waits[:-1]:
                        nop = mybir.InstNoOp(
                            name=f"I-waitnop-{k}", engine=eng, ins=[], outs=[],
                            sync_info=mybir.SyncInfo(on_wait=[w], on_update=[]))
                        k += 1
                        il.insert(idx, nop)
                        idx += 1
                idx += 1


def _prep_core_inputs(core, x, rope_cos, rope_sin, qkv_kernel, qkv_bias,
                      proj_kernel, proj_bias, q_norm_w, k_norm_w):
    b = core // 4
    heads = [3 * (core % 4) + i for i in range(HP)]

    wq = qkv_kernel.reshape(C, 3, H, HD)
    bq = qkv_bias.reshape(3, H, HD)

    xTa = np.ascontiguousarray(x[b].T).astype(BF)

    wqk = np.empty((C, HP * 128), np.float32)
    bqk = np.zeros((128, HP), np.float32)
    for i, h in enumerate(heads):
        wqk[:, i * 128:i * 128 + 64] = wq[:, 0, h, PERM]
        wqk[:, i * 128 + 64:(i + 1) * 128] = wq[:, 1, h, PERM]
        bqk[0:64, i] = bq[0, h, PERM]
        bqk[64:128, i] = bq[1, h, PERM]

    wv = np.zeros((C, HP * 64), np.float32)
    for i, h in enumerate(heads):
        wv[:, i * 64:(i + 1) * 64] = wq[:, 2, h, :]
    # packed [p, (c m)] so the SBUF copy is one dense DMA
    wvp = wv.reshape(CCH, 128, HP * 64).transpose(1, 0, 2).reshape(128, -1)

    cosT = rope_cos.T  # (HD, N)
    sinT = rope_sin.T
    cosw = np.empty((128, N), np.float32)
    sinw = np.empty((128, N), np.float32)
    cosw[0:64] = cosT[PERM] * q_norm_w[PERM][:, None]
    cosw[64:128] = cosT[PERM] * k_norm_w[PERM][:, None]
    # sin multiplies the SHUFFLED (partner) value -> partner's norm weight
    qn_p = q_norm_w[PERM][SWAPIDX]
    kn_p = k_norm_w[PERM][SWAPIDX]
    sinw[0:64] = SIGN[:, None] * sinT[PERM] * qn_p[:, None]
    sinw[64:128] = SIGN[:, None] * sinT[PERM] * kn_p[:, None]

    onesp = np.zeros((128, 2), np.float32)
    onesp[0:64, 0] = 1.0
    onesp[64:128, 1] = 1.0

    sel4 = np.zeros((128, 512), np.float32)
    for t in range(NT):
        sel4[32 * t, t * 128:t * 128 + 64] = 1.0
        sel4[32 * t + 1, t * 128 + 64:(t + 1) * 128] = 1.0

    rows01 = np.concatenate([np.arange(h * HD, (h + 1) * HD)
                             for h in heads[0:2]])
    rows2 = np.arange(heads[2] * HD, (heads[2] + 1) * HD)
    wp01 = proj_kernel[rows01, :]
    wp2 = proj_kernel[rows2, :]

    consts = np.zeros((128, 642), np.float32)
    consts[:, 0:2] = onesp
    consts[:, 2:514] = sel4
    consts[:, 514:642] = np.eye(128, dtype=np.float32)
    return {"xT": xTa, "wqk": wqk.astype(BF), "bqk": bqk,
            "cosw": cosw.astype(BF), "sinw": sinw.astype(BF),
            "wvp": np.ascontiguousarray(wvp).astype(BF),
            "wp01": np.ascontiguousarray(wp01).astype(BF),
            "wp2": np.ascontiguousarray(wp2).astype(BF),
            "consts": consts.astype(BF)}


def kernel(x, rope_cos, rope_sin, qkv_kernel, qkv_bias, proj_kernel,
           proj_bias, q_norm_w, k_norm_w, _trace=False):
    args = [np.asarray(a, dtype=np.float32) for a in
            (x, rope_cos, rope_sin, qkv_kernel, qkv_bias, proj_kernel,
             proj_bias, q_norm_w, k_norm_w)]
    in_maps = [_prep_core_inputs(c, *args) for c in range(NCORES)]

    if "nc" not in _NC_CACHE:
        _NC_CACHE["nc"] = build_nc()
    nc = _NC_CACHE["nc"]

    res = run_bass_kernel_spmd(nc, in_maps, core_ids=list(range(NCORES)),
                               trace=_trace)
    parts = [np.asarray(res.results[c]["out"]).astype(np.float32)
             for c in range(NCORES)]
    # v-bias contributes exactly bv @ proj_kernel (softmax rows sum to 1)
    pb = (np.asarray(proj_bias, dtype=np.float32)
          + np.asarray(qkv_bias, dtype=np.float32)[2 * C:]
          @ np.asarray(proj_kernel, dtype=np.float32))
    out = np.empty((B, N, C), np.float32)
    for b in range(B):
        out[b] = (parts[4 * b] + parts[4 * b + 1] + parts[4 * b + 2]
                  + parts[4 * b + 3] + pb)
    if _trace:
        kernel.last_results = res
    return out


# revision 10
# speedup vs baseline: 1.0039x; 1.0037x over previous
"""Multi-head attention (RMSNorm-QK + RoPE + softmax + proj) on 8 Trainium2 cores.

v2 design (cost-model-driven rewrite of the baseline):
 - bf16 operands everywhere (matmuls cost 1 cyc/row like fp32r, but DVE gets
   2x modes and DMA halves); fp32 PSUM accumulation throughout.
 - Transposed PV: O tiles are [128 q, 65] (64 dims + ones col for the softmax
   denominator), using all 128 output partitions -> PV drops from 32768 to
   16640 cyc/head, the denominator becomes a per-partition column (reciprocal
   + tensor_scalar mult), and the old broadcast-reciprocal matmuls vanish.
 - O^T for the projection comes from PE transposes (128 bf16 rows each).
 - RMS rsqrt via exp(-0.5 ln x) on ACT (same table as the softmax exp; the
   DVE has no pow/divide/rsqrt ISA), qkv bias added in the DVE pipeline
   (per-partition scalar), v bias folded into the host-side proj bias
   (softmax rows sum to 1), softmax denominators via batched DVE reciprocal.
 - RoPE elementwise work split DVE/Pool; emission order software-pipelines
   S(k+1) ahead of exp(k), stages a phase's px tiles in SBUF so each O
   qb-region accumulates contiguously (PSUM start bit stays per-element
   correct on HW), defers phase closes ~1.25 phases so early PE work (qkv+v)
   overlaps the ACT-bound exp stream, and pumps qkv/v/proj filler chunks into
   the PE gaps.

Sharding: core c handles batch c//4 and heads [3*(c%4), 3*(c%4)+3).
Each core writes a bf16 [N, C] partial; the host sums 4 partials per batch
and adds proj_bias + qkv_bias[v-part] @ proj_kernel.
"""
import sys

for _p in ("/opt/trn_rl_repo", "/opt/trn_rl_repo/concourse"):
    if _p not in sys.path:
        sys.path.insert(0, _p)

from collections import deque
from contextlib import ExitStack

import ml_dtypes
import numpy as np

import concourse.bass as bass
import concourse.mybir as mybir
import concourse.tile as tile
from concourse.bass_utils import run_bass_kernel_spmd

F32 = mybir.dt.float32
BF16 = mybir.dt.bfloat16
AF = mybir.ActivationFunctionType
ALU = mybir.AluOpType
BF = ml_dtypes.bfloat16

B, N, C = 2, 2048, 768
H, HD = 12, 64
HP = 3            # heads per core
NCORES = 8
CCH = 6           # contraction chunks of 128
NT = 4            # token tiles of 512
KB = 16           # k blocks of 128
NG = 8            # 2-kb groups per (head, qtile) phase

SWAP_MASK = [(i + 16) % 32 for i in range(32)]
PERM = np.concatenate([np.arange(0, 16), np.arange(32, 48),
                       np.arange(16, 32), np.arange(48, 64)])
SIGN = np.where(PERM < 32, -1.0, 1.0).astype(np.float32)
# rope partner of PERM-position p (SWAP_MASK's intra-32 half swap)
SWAPIDX = np.array([(p // 32) * 32 + (p + 16) % 32 for p in range(64)])

_NC_CACHE = {}


def build_nc(split_waits=True):
    nc = bass.Bass(target_bir_lowering=True)
    xT = nc.declare_dram_parameter("xT", [C, N], BF16, isOutput=False)
    wqk = nc.declare_dram_parameter("wqk", [C, HP * 128], BF16, isOutput=False)
    cosw = nc.declare_dram_parameter("cosw", [128, N], BF16, isOutput=False)
    sinw = nc.declare_dram_parameter("sinw", [128, N], BF16, isOutput=False)
    wvp = nc.declare_dram_parameter("wvp", [128, CCH * HP * 64], BF16,
                                    isOutput=False)
    wp01 = nc.declare_dram_parameter("wp01", [128, C], BF16, isOutput=False)
    wp2 = nc.declare_dram_parameter("wp2", [64, C], BF16, isOutput=False)
    # consts: [onesp(2) | sel4(512) | ident(128)]
    consts = nc.declare_dram_parameter("consts", [128, 642], BF16,
                                       isOutput=False)
    bqk = nc.declare_dram_parameter("bqk", [128, HP], F32, isOutput=False)
    out = nc.declare_dram_parameter("out", [N, C], BF16, isOutput=True)

    with tile.TileContext(nc) as tc, ExitStack() as ctx:
        sb = ctx.enter_context(tc.tile_pool(name="sb", bufs=1))
        pipe = ctx.enter_context(tc.tile_pool(name="pipe", bufs=2))
        pxp = pipe     # per-tag bufs below
        otp = pipe
        pop = pipe
        # PSUM: 4 + 2 + 1 + 1 = 8 banks, one pool with per-tag bufs
        sp = ctx.enter_context(tc.tile_pool(name="sp", bufs=2, space="PSUM"))
        qp = sp
        op = sp
        mp = sp

        # ---------- static SBUF tiles ----------
        xs = sb.tile([128, CCH, N], BF16, tag="xs")
        wqk_sb = sb.tile([128, CCH, HP * 128], BF16, tag="wqk")
        wv_sb = sb.tile([128, CCH, HP * 64], BF16, tag="wv")
        cos_sb = sb.tile([128, N], BF16, tag="cos")
        sin_sb = sb.tile([128, N], BF16, tag="sin")
        cn = sb.tile([128, 642], BF16, tag="cn")
        onesp_sb = cn[:, 0:2]
        sel_sb = cn[:, 2:514]
        ident_sb = cn[:, 514:642]
        bqk_sb = sb.tile([128, HP], F32, tag="bqk")
        wp01_sb = sb.tile([128, C], BF16, tag="wp01")
        wp2_sb = sb.tile([64, C], BF16, tag="wp2")

        q12 = sb.tile([128, N], BF16, tag="q12")
        k12 = sb.tile([128, N], BF16, tag="k12")
        q3 = sb.tile([64, N], BF16, tag="q3")
        k3 = sb.tile([64, N], BF16, tag="k3")
        t4_all = sb.tile([128, N], BF16, tag="t4_all")
        s_sb = sb.tile([128, 512], F32, tag="s_sb")
        sv = sb.tile([128, 512], BF16, tag="sv")
        v3 = sb.tile([128, KB, HP, 65], BF16, tag="v3")
        ones48 = sb.tile([128, KB * HP], BF16, tag="ones48")
        o2 = sb.tile([128, NT, 4, 128], BF16, tag="o2")
        o1 = sb.tile([128, NT, 4, 64], BF16, tag="o1")

        def qT(h):
            return (q12[0:64], q12[64:128], q3[:])[h]

        def kT(h):
            return (k12[0:64], k12[64:128], k3[:])[h]

        # ---------- prologue DMAs (ordered for earliest qkv start) ----------
        xT_r = xT[:].rearrange("(c p) n -> p c n", p=128)
        wqk_r = wqk[:].rearrange("(c p) m -> p c m", p=128)
        d = nc.sync.dma_start
        d(wqk_sb[:, 0:2, :], wqk_r[:, 0:2, :])
        d(xs[:, 0:3, 0:512], xT_r[:, 0:3, 0:512])      # tile-0 tokens
        d(wqk_sb[:, 2:6, :], wqk_r[:, 2:6, :])
        d(xs[:, 3:6, 0:512], xT_r[:, 3:6, 0:512])
        d(bqk_sb[:], bqk[:, :])
        d(cn[:], consts[:, :])
        d(cos_sb[:, 0:1024], cosw[:, 0:1024])
        d(sin_sb[:, 0:1024], sinw[:, 0:1024])
        d(xs[:, :, 512:1024], xT_r[:, :, 512:1024])
        d(xs[:, :, 1024:1536], xT_r[:, :, 1024:1536])
        d(wv_sb[:].rearrange("p c m -> p (c m)"), wvp[:, :])
        d(xs[:, :, 1536:2048], xT_r[:, :, 1536:2048])
        d(cos_sb[:, 1024:2048], cosw[:, 1024:2048])
        d(sin_sb[:, 1024:2048], sinw[:, 1024:2048])
        d(wp01_sb[:], wp01[:, :])
        d(wp2_sb[:], wp2[:, :])

        nc.vector.memset(sv[:], 1.0)   # rows never written stay 1 (sel zeros them)
        nc.vector.memset(s_sb[:], 1.0)
        nc.vector.memset(ones48[:], 1.0)
        nc.vector.tensor_copy(
            v3[:].rearrange("p a b n -> p (a b) n", n=65)[:, :, 64], ones48[:])

        def mm(out_ap, lhsT, rhs, start, stop):
            nc.tensor.matmul(out_ap, lhsT, rhs, start=start, stop=stop,
                             skip_group_check=True)

        # ---------- qkv-head generator ----------
        # Per-tile chunks: mms -> RoPE pipe -> finA (sumsq+rsqrt) ->
        # finB (broadcast+scale). Emission defers fins so PE stays dense;
        # the qp ring (2) tolerates exactly one deferred finA.
        def qkv_gen(h):
            hs = slice(h * 128, (h + 1) * 128)
            qk = [None] * NT

            def mms(t):
                ts = slice(t * 512, (t + 1) * 512)
                qk[t] = qp.tile([128, 512], F32, tag="q", name=f"qk{t}")
                for c in range(CCH):
                    mm(qk[t][:], wqk_sb[:, c, hs], xs[:, c, ts], c == 0,
                       c == CCH - 1)

            def rope(t):
                ts = slice(t * 512, (t + 1) * 512)
                qkb = pipe.tile([128, 512], BF16, tag="qkb")
                nc.vector.tensor_scalar(qkb[:], qk[t][:], bqk_sb[:, h:h + 1],
                                        None, ALU.add)
                sq = pipe.tile([128, 512], BF16, tag="sq")
                if h == 0:
                    nc.vector.tensor_mul(sq[:], qkb[:], qkb[:])
                t1 = pipe.tile([128, 512], BF16, tag="t1")
                nc.gpsimd.tensor_mul(t1[:], qkb[:], cos_sb[:, ts])
                t2 = pipe.tile([128, 512], BF16, tag="t2")
                nc.vector.stream_shuffle(t2[:], qkb[:], SWAP_MASK)
                t3 = pipe.tile([128, 512], BF16, tag="t3")
                nc.vector.tensor_mul(t3[:], t2[:], sin_sb[:, ts])
                nc.vector.tensor_add(t4_all[:, ts], t1[:], t3[:])
                if h != 0:
                    nc.gpsimd.tensor_mul(sq[:], qkb[:], qkb[:])
                return sq

            def finA(t, sq):
                rows = slice(32 * t, 32 * t + 2)
                mm(qk[t][0:2, :], onesp_sb[:], sq[:], True, True)
                if h == 0:
                    # rsqrt = exp(-0.5 ln(ms)); same ACT table as softmax exp
                    lv = pipe.tile([2, 512], F32, tag="lv", name="lv")
                    nc.scalar.activation(lv[:], qk[t][0:2, :], AF.Ln,
                                         bias=0.0, scale=1.0 / HD)
                    nc.scalar.activation(sv[rows, :], lv[:], AF.Exp,
                                         bias=0.0, scale=-0.5)
                else:
                    nc.vector.tensor_copy(s_sb[rows, :], qk[t][0:2, :])

            def lnexp():
                lva = pipe.tile([128, 512], F32, tag="lva", name="lva")
                nc.scalar.activation(lva[:], s_sb[:], AF.Ln,
                                     bias=0.0, scale=1.0 / HD)
                nc.scalar.activation(sv[:], lva[:], AF.Exp, bias=0.0,
                                     scale=-0.5)

            def finB(t):
                ts = slice(t * 512, (t + 1) * 512)
                sqk_ps = qp.tile([128, 512], F32, tag="q")
                mm(sqk_ps[:], sel_sb[:, t * 128:(t + 1) * 128], sv[:],
                   True, True)
                nc.vector.tensor_mul(qT(h)[:, ts], t4_all[0:64, ts],
                                     sqk_ps[0:64, :])
                nc.vector.tensor_mul(kT(h)[:, ts], t4_all[64:128, ts],
                                     sqk_ps[64:128, :])

            sqs = [None] * NT

            def do_mms(t):
                mms(t)
                sqs[t] = rope(t)

            do_mms(0)
            yield 4500
            do_mms(1)
            yield 4500
            finA(0, sqs[0])
            yield 700
            do_mms(2)
            yield 4500
            finA(1, sqs[1])
            if h == 0:
                finB(0)
            yield 1600
            finA(2, sqs[2])
            yield 700
            do_mms(3)
            yield 4500
            if h == 0:
                finB(1)
            yield 900
            finA(3, sqs[3])
            yield 700
            if h != 0:
                lnexp()
                yield 700
                finB(0)
                yield 900
                finB(1)
                yield 900
            finB(2)
            yield 900
            finB(3)
            yield 900

        # ---------- v generator ----------
        def v_gen():
            for tt in range(KB):
                v_ps = qp.tile([128, HP * 64], F32, tag="q")
                for c in range(CCH):
                    mm(v_ps[:], xs[:, c, tt * 128:(tt + 1) * 128],
                       wv_sb[:, c, :], c == 0, c == CCH - 1)
                nc.vector.tensor_copy(
                    v3[:, tt, :, 0:64],
                    v_ps[:, :].rearrange("p (h n) -> p h n", h=HP))
                yield 1500

        # ---------- proj of one (qtile, qblock) ----------
        mtr = [None]

        def proj_qb(qt, qb):
            if mtr[0] is None:
                mtr[0] = mp.tile([128, 4, 128], F32, tag="m", name="mtr", bufs=1)
            m = mtr[0]
            tr01 = m[:, qb, 0:64].bitcast(BF16)
            tr2 = m[0:64, qb, 64:128].bitcast(BF16)
            nc.tensor.transpose(tr01, o2[:, qt, qb, :], ident_sb[:])
            nc.tensor.transpose(tr2, o1[:, qt, qb, :], ident_sb[:])
            on_act = qt == 3   # ACT is idle once the last exps drain
            ot01 = otp.tile([128, 128], BF16, tag="ot01", bufs=6)
            ot2 = otp.tile([64, 128], BF16, tag="ot2", bufs=6)
            if on_act:
                nc.scalar.activation(ot01[:], tr01, AF.Copy, bias=0.0,
                                     scale=1.0)
                nc.vector.tensor_copy(ot2[:], tr2)
            else:
                nc.vector.tensor_copy(ot01[:], tr01)
                nc.vector.tensor_copy(ot2[:], tr2)
            po = pop.tile([128, C], BF16, tag="po", bufs=3)
            for half in range(2):
                cs = slice(half * 384, (half + 1) * 384)
                p_ps = qp.tile([128, 384], F32, tag="q")
                mm(p_ps[:], ot01[:], wp01_sb[:, cs], True, False)
                mm(p_ps[:], ot2[:], wp2_sb[:, cs], False, True)
                if on_act and half == 1:
                    nc.scalar.activation(po[:, cs], p_ps[:], AF.Copy,
                                         bias=0.0, scale=1.0)
                else:
                    nc.vector.tensor_copy(po[:, cs], p_ps[:])
            tb = qt * 4 + qb
            nc.sync.dma_start(out[tb * 128:(tb + 1) * 128, :], po[:])

        # ---------- filler pump ----------
        fillers = deque()
        debt = [0.0]

        def pump(budget):
            budget += debt[0]
            while budget > 0 and fillers:
                try:
                    budget -= next(fillers[0])
                except StopIteration:
                    fillers.popleft()
            debt[0] = min(budget, 3000.0)

        def ensure_done(gen):
            """Pump until `gen` has fully emitted (emission-order guard for
            cross-generator data deps)."""
            while gen in fillers:
                pump(100000)

        # ---------- attention stream ----------
        phases = [(h, qt) for h in range(HP) for qt in range(NT)]
        px_tiles = {}
        emitted = set()

        def emit_group(p, g):
            if (p, g) in emitted:
                return
            emitted.add((p, g))
            h, qt = phases[p]
            qs = slice(qt * 512, (qt + 1) * 512)
            s_ps = sp.tile([128, 1024], F32, tag="s")
            for j in range(2):
                kb = 2 * g + j
                mm(s_ps[:, j * 512:(j + 1) * 512],
                   kT(h)[:, kb * 128:(kb + 1) * 128], qT(h)[:, qs],
                   True, True)
            px = pxp.tile([128, 1024], BF16, tag="px", bufs=28)
            nc.scalar.activation(px[:], s_ps[:], AF.Exp, bias=0.0, scale=0.125)
            px_tiles[(p, g)] = px

        def close_gen(p):
            if p == 0:
                ensure_done(vg)   # PV reads v3; emission-order guard
            h, qt = phases[p]
            o_ps = op.tile([128, 4, 65], F32, tag="o", bufs=1)
            for qb in range(4):
                for g in range(NG):
                    px = px_tiles[(p, g)]
                    for j in range(2):
                        kb = 2 * g + j
                        mm(o_ps[:, qb, :],
                           px[:, j * 512 + qb * 128:j * 512 + (qb + 1) * 128],
                           v3[:, kb, h, :],
                           qb == 0 and kb == 0, kb == KB - 1)
                if qb == 1 or qb == 3:
                    yield
            # normalize by the ones-column denominators (batched reciprocal,
            # then per-qb per-partition multiply); epilogues after ALL PV so
            # coarse WAR tracking can't serialize the qb bundles
            rec4 = pipe.tile([128, 4], F32, tag="rec4", name="rec4")
            nc.vector.reciprocal(rec4[:], o_ps[:, :, 64])
            for qb in range(4):
                dst = (o2[:, qt, qb, h * 64:(h + 1) * 64] if h < 2
                       else o1[:, qt, qb, :])
                nc.vector.tensor_scalar(dst, o_ps[:, qb, 0:64],
                                        rec4[:, qb:qb + 1], None, ALU.mult)
                if h == 2:
                    proj_qb(qt, qb)
                yield
            for g in range(NG):
                del px_tiles[(p, g)]

        def drain_close(cg):
            for _ in cg:
                pass

        # ---------- main schedule ----------
        # Phase 0 runs with qkv(h0) inlined per tile: tile t unlocks S groups
        # 2t, 2t+1 (k-tiles) while qT(qt0) comes entirely from tile 0.
        qg0 = qkv_gen(0)
        vg = v_gen()
        g1, g2 = qkv_gen(1), qkv_gen(2)
        fillers.append(vg)
        fillers.append(g1)
        need_gen = {1: g1, 2: g2}

        def drain_n(gen, n_chunks):
            for _ in range(n_chunks):
                next(gen)

        # Front: dense qkv-h0 mms with the exp stream fed by EVERY group
        # whose gates are open. Tile t gates k-blocks 4t..4t+3 (groups
        # 2t,2t+1 of every h0 phase) and the q-tokens of phase (0,t).
        drain_n(qg0, 5)              # thru finB(0)
        front = [(0, 0), (0, 1),
                 "T1", (0, 2), (0, 3), (1, 0), (1, 1),
                 "T2", (0, 4), (0, 5), (1, 2), (1, 3), (2, 0), (2, 1),
                 "T3", (0, 6), (0, 7), (1, 4), (1, 5), (2, 2), (2, 3),
                 (3, 0), (3, 1)]
        for item in front:
            if item == "T1":
                drain_n(qg0, 3)      # finA2, mms3, finB1
            elif item == "T2":
                drain_n(qg0, 2)      # finA3, finB2
            elif item == "T3":
                drain_n(qg0, 1)      # finB3
            else:
                emit_group(*item)
                pump(600)

        # Steady state: early closes deferred ~2 phases (px ring holds ~3
        # phases) so v/qkv fillers use the early PE slack; late closes pulled
        # in so proj work overlaps the remaining exp stream.
        NP = len(phases)
        close_at = {}
        for p in range(NP - 1):
            # early closes deferred 3 phases (shifts PV out of the PE-heavy
            # qkv/v window); h2 closes pulled in so proj overlaps exps
            lag = (p + 3, 0) if p < 8 else (p + 1, 0)
            close_at.setdefault(lag, []).append(p)

        def after_close(cp):
            if phases[cp] == (0, 2):
                fillers.append(g2)

        active_closes = []
        for p in range(1, NP):
            nh = phases[p][0]
            if nh != phases[p - 1][0]:
                ensure_done(need_gen[nh])
            for g in range(NG):
                emit_group(p, g)
                pump(2400)
                for cp in close_at.get((p, g), []):
                    active_closes.append((cp, close_gen(cp)))
                if active_closes:
                    steps = 2 if p >= NP - 2 else 1
                    for _ in range(steps):
                        if not active_closes:
                            break
                        cp, cg = active_closes[0]
                        try:
                            next(cg)
                        except StopIteration:
                            active_closes.pop(0)
                            after_close(cp)
        for cp, cg in active_closes:
            drain_close(cg)
            after_close(cp)
        drain_close(close_gen(NP - 1))
        while fillers:
            pump(100000)

    if split_waits:
        _split_waits(nc)
    return nc


def _split_waits(nc):
    """Walrus lowers at most one sync-wait per instruction; move excess waits
    onto NoOps inserted just before, on the same engine queue."""
    k = 0
    for fn in nc.m.functions:
        for bb in fn.blocks:
            il = bb.instructions
            idx = 0
            while idx < len(il):
                inst = il[idx]
                si = inst.sync_info
                eng = getattr(inst, "engine", None)
                if (si is not None and len(si.on_wait) > 1
                        and eng is not None
                        and str(eng) != "EngineType.Unassigned"):
                    waits = list(si.on_wait)
                    inst.sync_info = mybir.SyncInfo(
                        on_wait=[waits[-1]], on_update=list(si.on_update))
                    for w in waits[:-1]:
                        nop = mybir.InstNoOp(
                            name=f"I-waitnop-{k}", engine=eng, ins=[], outs=[],
                            sync_info=mybir.SyncInfo(on_wait=[w], on_update=[]))
                        k += 1
                        il.insert(idx, nop)
                        idx += 1
                idx += 1


def _prep_core_inputs(core, x, rope_cos, rope_sin, qkv_kernel, qkv_bias,
                      proj_kernel, proj_bias, q_norm_w, k_norm_w):
    b = core // 4
    heads = [3 * (core % 4) + i for i in range(HP)]

    wq = qkv_kernel.reshape(C, 3, H, HD)
    bq = qkv_bias.reshape(3, H, HD)

    xTa = np.ascontiguousarray(x[b].T).astype(BF)

    wqk = np.empty((C, HP * 128), np.float32)
    bqk = np.zeros((128, HP), np.float32)
    for i, h in enumerate(heads):
        wqk[:, i * 128:i * 128 + 64] = wq[:, 0, h, PERM]
        wqk[:, i * 128 + 64:(i + 1) * 128] = wq[:, 1, h, PERM]
        bqk[0:64, i] = bq[0, h, PERM]
        bqk[64:128, i] = bq[1, h, PERM]

    wv = np.zeros((C, HP * 64), np.float32)
    for i, h in enumerate(heads):
        wv[:, i * 64:(i + 1) * 64] = wq[:, 2, h, :]
    # packed [p, (c m)] so the SBUF copy is one dense DMA
    wvp = wv.reshape(CCH, 128, HP * 64).transpose(1, 0, 2).reshape(128, -1)

    cosT = rope_cos.T  # (HD, N)
    sinT = rope_sin.T
    cosw = np.empty((128, N), np.float32)
    sinw = np.empty((128, N), np.float32)
    cosw[0:64] = cosT[PERM] * q_norm_w[PERM][:, None]
    cosw[64:128] = cosT[PERM] * k_norm_w[PERM][:, None]
    # sin multiplies the SHUFFLED (partner) value -> partner's norm weight
    qn_p = q_norm_w[PERM][SWAPIDX]
    kn_p = k_norm_w[PERM][SWAPIDX]
    sinw[0:64] = SIGN[:, None] * sinT[PERM] * qn_p[:, None]
    sinw[64:128] = SIGN[:, None] * sinT[PERM] * kn_p[:, None]

    onesp = np.zeros((128, 2), np.float32)
    onesp[0:64, 0] = 1.0
    onesp[64:128, 1] = 1.0

    sel4 = np.zeros((128, 512), np.float32)
    for t in range(NT):
        sel4[32 * t, t * 128:t * 128 + 64] = 1.0
        sel4[32 * t + 1, t * 128 + 64:(t + 1) * 128] = 1.0

    rows01 = np.concatenate([np.arange(h * HD, (h + 1) * HD)
                             for h in heads[0:2]])
    rows2 = np.arange(heads[2] * HD, (heads[2] + 1) * HD)
    wp01 = proj_kernel[rows01, :]
    wp2 = proj_kernel[rows2, :]

    consts = np.zeros((128, 642), np.float32)
    consts[:, 0:2] = onesp
    consts[:, 2:514] = sel4
    consts[:, 514:642] = np.eye(128, dtype=np.float32)
    return {"xT": xTa, "wqk": wqk.astype(BF), "bqk": bqk,
            "cosw": cosw.astype(BF), "sinw": sinw.astype(BF),
            "wvp": np.ascontiguousarray(wvp).astype(BF),
            "wp01": np.ascontiguousarray(wp01).astype(BF),
            "wp2": np.ascontiguousarray(wp2).astype(BF),
            "consts": consts.astype(BF)}


def kernel(x, rope_cos, rope_sin, qkv_kernel, qkv_bias, proj_kernel,
           proj_bias, q_norm_w, k_norm_w, _trace=False):
    args = [np.asarray(a, dtype=np.float32) for a in
            (x, rope_cos, rope_sin, qkv_kernel, qkv_bias, proj_kernel,
             proj_bias, q_norm_w, k_norm_w)]
    in_maps = [_prep_core_inputs(c, *args) for c in range(NCORES)]

    if "nc" not in _NC_CACHE:
        _NC_CACHE["nc"] = build_nc()
    nc = _NC_CACHE["nc"]

    res = run_bass_kernel_spmd(nc, in_maps, core_ids=list(range(NCORES)),
                               trace=_trace)
    parts = [np.asarray(res.results[c]["out"]).astype(np.float32)
             for c in range(NCORES)]
    # v-bias contributes exactly bv @ proj_kernel (softmax rows sum to 1)
    pb = (np.asarray(proj_bias, dtype=np.float32)
          + np.asarray(qkv_bias, dtype=np.float32)[2 * C:]
          @ np.asarray(proj_kernel, dtype=np.float32))
    out = np.empty((B, N, C), np.float32)
    for b in range(B):
        out[b] = (parts[4 * b] + parts[4 * b + 1] + parts[4 * b + 2]
                  + parts[4 * b + 3] + pb)
    if _trace:
        kernel.last_results = res
    return out


# revision 11
# speedup vs baseline: 1.0079x; 1.0040x over previous
"""Multi-head attention (RMSNorm-QK + RoPE + softmax + proj) on 8 Trainium2 cores.

v2 design (cost-model-driven rewrite of the baseline):
 - bf16 operands everywhere (matmuls cost 1 cyc/row like fp32r, but DVE gets
   2x modes and DMA halves); fp32 PSUM accumulation throughout.
 - Transposed PV: O tiles are [128 q, 65] (64 dims + ones col for the softmax
   denominator), using all 128 output partitions -> PV drops from 32768 to
   16640 cyc/head, the denominator becomes a per-partition column (reciprocal
   + tensor_scalar mult), and the old broadcast-reciprocal matmuls vanish.
 - O^T for the projection comes from PE transposes (128 bf16 rows each).
 - RMS rsqrt via exp(-0.5 ln x) on ACT (same table as the softmax exp; the
   DVE has no pow/divide/rsqrt ISA), qkv bias added in the DVE pipeline
   (per-partition scalar), v bias folded into the host-side proj bias
   (softmax rows sum to 1), softmax denominators via batched DVE reciprocal.
 - RoPE elementwise work split DVE/Pool; emission order software-pipelines
   S(k+1) ahead of exp(k), stages a phase's px tiles in SBUF so each O
   qb-region accumulates contiguously (PSUM start bit stays per-element
   correct on HW), defers phase closes ~1.25 phases so early PE work (qkv+v)
   overlaps the ACT-bound exp stream, and pumps qkv/v/proj filler chunks into
   the PE gaps.

Sharding: core c handles batch c//4 and heads [3*(c%4), 3*(c%4)+3).
Each core writes a bf16 [N, C] partial; the host sums 4 partials per batch
and adds proj_bias + qkv_bias[v-part] @ proj_kernel.
"""
import sys

for _p in ("/opt/trn_rl_repo", "/opt/trn_rl_repo/concourse"):
    if _p not in sys.path:
        sys.path.insert(0, _p)

from collections import deque
from contextlib import ExitStack

import ml_dtypes
import numpy as np

import concourse.bass as bass
import concourse.mybir as mybir
import concourse.tile as tile
from concourse.bass_utils import run_bass_kernel_spmd

F32 = mybir.dt.float32
BF16 = mybir.dt.bfloat16
AF = mybir.ActivationFunctionType
ALU = mybir.AluOpType
BF = ml_dtypes.bfloat16

B, N, C = 2, 2048, 768
H, HD = 12, 64
HP = 3            # heads per core
NCORES = 8
CCH = 6           # contraction chunks of 128
NT = 4            # token tiles of 512
KB = 16           # k blocks of 128
NG = 8            # 2-kb groups per (head, qtile) phase

SWAP_MASK = [(i + 16) % 32 for i in range(32)]
PERM = np.concatenate([np.arange(0, 16), np.arange(32, 48),
                       np.arange(16, 32), np.arange(48, 64)])
SIGN = np.where(PERM < 32, -1.0, 1.0).astype(np.float32)
# rope partner of PERM-position p (SWAP_MASK's intra-32 half swap)
SWAPIDX = np.array([(p // 32) * 32 + (p + 16) % 32 for p in range(64)])

_NC_CACHE = {}


def build_nc(split_waits=True):
    nc = bass.Bass(target_bir_lowering=True)
    xT = nc.declare_dram_parameter("xT", [C, N], BF16, isOutput=False)
    wqk = nc.declare_dram_parameter("wqk", [C, HP * 128], BF16, isOutput=False)
    cosw = nc.declare_dram_parameter("cosw", [128, N], BF16, isOutput=False)
    sinw = nc.declare_dram_parameter("sinw", [128, N], BF16, isOutput=False)
    wvp = nc.declare_dram_parameter("wvp", [128, CCH * HP * 64], BF16,
                                    isOutput=False)
    wp01 = nc.declare_dram_parameter("wp01", [128, C], BF16, isOutput=False)
    wp2 = nc.declare_dram_parameter("wp2", [64, C], BF16, isOutput=False)
    # consts: [onesp(2) | sel4(512) | ident(128)]
    consts = nc.declare_dram_parameter("consts", [128, 642], BF16,
                                       isOutput=False)
    bqk = nc.declare_dram_parameter("bqk", [128, HP], F32, isOutput=False)
    out = nc.declare_dram_parameter("out", [N, C], BF16, isOutput=True)

    with tile.TileContext(nc) as tc, ExitStack() as ctx:
        sb = ctx.enter_context(tc.tile_pool(name="sb", bufs=1))
        pipe = ctx.enter_context(tc.tile_pool(name="pipe", bufs=2))
        pxp = pipe     # per-tag bufs below
        otp = pipe
        pop = pipe
        # PSUM: 4 + 2 + 1 + 1 = 8 banks, one pool with per-tag bufs
        sp = ctx.enter_context(tc.tile_pool(name="sp", bufs=2, space="PSUM"))
        qp = sp
        op = sp
        mp = sp

        # ---------- static SBUF tiles ----------
        xs = sb.tile([128, CCH, N], BF16, tag="xs")
        wqk_sb = sb.tile([128, CCH, HP * 128], BF16, tag="wqk")
        wv_sb = sb.tile([128, CCH, HP * 64], BF16, tag="wv")
        cos_sb = sb.tile([128, N], BF16, tag="cos")
        sin_sb = sb.tile([128, N], BF16, tag="sin")
        cn = sb.tile([128, 642], BF16, tag="cn")
        onesp_sb = cn[:, 0:2]
        sel_sb = cn[:, 2:514]
        ident_sb = cn[:, 514:642]
        bqk_sb = sb.tile([128, HP], F32, tag="bqk")
        wp01_sb = sb.tile([128, C], BF16, tag="wp01")
        wp2_sb = sb.tile([64, C], BF16, tag="wp2")

        q12 = sb.tile([128, N], BF16, tag="q12")
        k12 = sb.tile([128, N], BF16, tag="k12")
        q3 = sb.tile([64, N], BF16, tag="q3")
        k3 = sb.tile([64, N], BF16, tag="k3")
        t4_all = sb.tile([128, N], BF16, tag="t4_all")
        s_sb = sb.tile([128, 512], F32, tag="s_sb")
        sv = sb.tile([128, 512], BF16, tag="sv")
        v3 = sb.tile([128, KB, HP, 65], BF16, tag="v3")
        ones48 = sb.tile([128, KB * HP], BF16, tag="ones48")
        o2 = sb.tile([128, NT, 4, 128], BF16, tag="o2")
        o1 = sb.tile([128, NT, 4, 64], BF16, tag="o1")

        def qT(h):
            return (q12[0:64], q12[64:128], q3[:])[h]

        def kT(h):
            return (k12[0:64], k12[64:128], k3[:])[h]

        # ---------- prologue DMAs (ordered for earliest qkv start) ----------
        xT_r = xT[:].rearrange("(c p) n -> p c n", p=128)
        wqk_r = wqk[:].rearrange("(c p) m -> p c m", p=128)
        d = nc.sync.dma_start
        d(wqk_sb[:, 0:2, :], wqk_r[:, 0:2, :])
        d(xs[:, 0:3, 0:512], xT_r[:, 0:3, 0:512])      # tile-0 tokens
        d(wqk_sb[:, 2:6, :], wqk_r[:, 2:6, :])
        d(xs[:, 3:6, 0:512], xT_r[:, 3:6, 0:512])
        d(bqk_sb[:], bqk[:, :])
        d(cn[:], consts[:, :])
        d(cos_sb[:, 0:1024], cosw[:, 0:1024])
        d(sin_sb[:, 0:1024], sinw[:, 0:1024])
        d(xs[:, :, 512:1024], xT_r[:, :, 512:1024])
        d(xs[:, :, 1024:1536], xT_r[:, :, 1024:1536])
        d(wv_sb[:].rearrange("p c m -> p (c m)"), wvp[:, :])
        d(xs[:, :, 1536:2048], xT_r[:, :, 1536:2048])
        d(cos_sb[:, 1024:2048], cosw[:, 1024:2048])
        d(sin_sb[:, 1024:2048], sinw[:, 1024:2048])
        d(wp01_sb[:], wp01[:, :])
        d(wp2_sb[:], wp2[:, :])

        nc.vector.memset(sv[:], 1.0)   # rows never written stay 1 (sel zeros them)
        nc.vector.memset(s_sb[:], 1.0)
        nc.vector.memset(ones48[:], 1.0)
        nc.vector.tensor_copy(
            v3[:].rearrange("p a b n -> p (a b) n", n=65)[:, :, 64], ones48[:])

        def mm(out_ap, lhsT, rhs, start, stop):
            nc.tensor.matmul(out_ap, lhsT, rhs, start=start, stop=stop,
                             skip_group_check=True)

        # ---------- qkv-head generator ----------
        # Per-tile chunks: mms -> RoPE pipe -> finA (sumsq+rsqrt) ->
        # finB (broadcast+scale). Emission defers fins so PE stays dense;
        # the qp ring (2) tolerates exactly one deferred finA.
        def qkv_gen(h):
            hs = slice(h * 128, (h + 1) * 128)
            qk = [None] * NT

            def mms(t):
                ts = slice(t * 512, (t + 1) * 512)
                qk[t] = qp.tile([128, 512], F32, tag="q", name=f"qk{t}")
                for c in range(CCH):
                    mm(qk[t][:], wqk_sb[:, c, hs], xs[:, c, ts], c == 0,
                       c == CCH - 1)

            def rope(t):
                ts = slice(t * 512, (t + 1) * 512)
                qkb = pipe.tile([128, 512], BF16, tag="qkb")
                nc.vector.tensor_scalar(qkb[:], qk[t][:], bqk_sb[:, h:h + 1],
                                        None, ALU.add)
                sq = pipe.tile([128, 512], BF16, tag="sq")
                if h == 0:
                    nc.vector.tensor_mul(sq[:], qkb[:], qkb[:])
                t1 = pipe.tile([128, 512], BF16, tag="t1")
                nc.gpsimd.tensor_mul(t1[:], qkb[:], cos_sb[:, ts])
                t2 = pipe.tile([128, 512], BF16, tag="t2")
                nc.vector.stream_shuffle(t2[:], qkb[:], SWAP_MASK)
                t3 = pipe.tile([128, 512], BF16, tag="t3")
                nc.vector.tensor_mul(t3[:], t2[:], sin_sb[:, ts])
                nc.vector.tensor_add(t4_all[:, ts], t1[:], t3[:])
                if h != 0:
                    nc.gpsimd.tensor_mul(sq[:], qkb[:], qkb[:])
                return sq

            def finA(t, sq):
                rows = slice(32 * t, 32 * t + 2)
                mm(qk[t][0:2, :], onesp_sb[:], sq[:], True, True)
                if h == 0:
                    # rsqrt = exp(-0.5 ln(ms)); same ACT table as softmax exp
                    lv = pipe.tile([2, 512], F32, tag="lv", name="lv")
                    nc.scalar.activation(lv[:], qk[t][0:2, :], AF.Ln,
                                         bias=0.0, scale=1.0 / HD)
                    nc.scalar.activation(sv[rows, :], lv[:], AF.Exp,
                                         bias=0.0, scale=-0.5)
                else:
                    nc.vector.tensor_copy(s_sb[rows, :], qk[t][0:2, :])

            def lnexp():
                lva = pipe.tile([128, 512], F32, tag="lva", name="lva")
                nc.scalar.activation(lva[:], s_sb[:], AF.Ln,
                                     bias=0.0, scale=1.0 / HD)
                nc.scalar.activation(sv[:], lva[:], AF.Exp, bias=0.0,
                                     scale=-0.5)

            def finB(t):
                ts = slice(t * 512, (t + 1) * 512)
                sqk_ps = qp.tile([128, 512], F32, tag="q")
                mm(sqk_ps[:], sel_sb[:, t * 128:(t + 1) * 128], sv[:],
                   True, True)
                nc.vector.tensor_mul(qT(h)[:, ts], t4_all[0:64, ts],
                                     sqk_ps[0:64, :])
                nc.vector.tensor_mul(kT(h)[:, ts], t4_all[64:128, ts],
                                     sqk_ps[64:128, :])

            sqs = [None] * NT

            def do_mms(t):
                mms(t)
                sqs[t] = rope(t)

            do_mms(0)
            yield 4500
            do_mms(1)
            yield 4500
            finA(0, sqs[0])
            yield 700
            do_mms(2)
            yield 4500
            finA(1, sqs[1])
            if h == 0:
                finB(0)
            yield 1600
            finA(2, sqs[2])
            yield 700
            do_mms(3)
            yield 4500
            if h == 0:
                finB(1)
            yield 900
            finA(3, sqs[3])
            yield 700
            if h != 0:
                lnexp()
                yield 700
                finB(0)
                yield 900
                finB(1)
                yield 900
            finB(2)
            yield 900
            finB(3)
            yield 900

        # ---------- v generator ----------
        def v_gen():
            for tt in range(KB):
                v_ps = qp.tile([128, HP * 64], F32, tag="q")
                for c in range(CCH):
                    mm(v_ps[:], xs[:, c, tt * 128:(tt + 1) * 128],
                       wv_sb[:, c, :], c == 0, c == CCH - 1)
                nc.vector.tensor_copy(
                    v3[:, tt, :, 0:64],
                    v_ps[:, :].rearrange("p (h n) -> p h n", h=HP))
                yield 1500

        # ---------- proj of one (qtile, qblock) ----------
        mtr = [None]

        def proj_qb(qt, qb):
            if mtr[0] is None:
                mtr[0] = mp.tile([128, 4, 128], F32, tag="m", name="mtr", bufs=1)
            m = mtr[0]
            tr01 = m[:, qb, 0:64].bitcast(BF16)
            tr2 = m[0:64, qb, 64:128].bitcast(BF16)
            nc.tensor.transpose(tr01, o2[:, qt, qb, :], ident_sb[:])
            nc.tensor.transpose(tr2, o1[:, qt, qb, :], ident_sb[:])
            on_act = qt == 3   # ACT is idle once the last exps drain
            ot01 = otp.tile([128, 128], BF16, tag="ot01", bufs=6)
            ot2 = otp.tile([64, 128], BF16, tag="ot2", bufs=6)
            if on_act:
                nc.scalar.activation(ot01[:], tr01, AF.Copy, bias=0.0,
                                     scale=1.0)
                nc.vector.tensor_copy(ot2[:], tr2)
            else:
                nc.vector.tensor_copy(ot01[:], tr01)
                nc.vector.tensor_copy(ot2[:], tr2)
            po = pop.tile([128, C], BF16, tag="po", bufs=3)
            for half in range(2):
                cs = slice(half * 384, (half + 1) * 384)
                p_ps = qp.tile([128, 384], F32, tag="q")
                mm(p_ps[:], ot01[:], wp01_sb[:, cs], True, False)
                mm(p_ps[:], ot2[:], wp2_sb[:, cs], False, True)
                if on_act and half == 1:
                    nc.scalar.activation(po[:, cs], p_ps[:], AF.Copy,
                                         bias=0.0, scale=1.0)
                else:
                    nc.vector.tensor_copy(po[:, cs], p_ps[:])
            tb = qt * 4 + qb
            nc.sync.dma_start(out[tb * 128:(tb + 1) * 128, :], po[:])

        # ---------- filler pump ----------
        fillers = deque()
        debt = [0.0]

        def pump(budget):
            budget += debt[0]
            while budget > 0 and fillers:
                try:
                    budget -= next(fillers[0])
                except StopIteration:
                    fillers.popleft()
            debt[0] = min(budget, 3000.0)

        def ensure_done(gen):
            """Pump until `gen` has fully emitted (emission-order guard for
            cross-generator data deps)."""
            while gen in fillers:
                pump(100000)

        # ---------- attention stream ----------
        phases = [(h, qt) for h in range(HP) for qt in range(NT)]
        px_tiles = {}
        emitted = set()

        def emit_group(p, g):
            if (p, g) in emitted:
                return
            emitted.add((p, g))
            h, qt = phases[p]
            qs = slice(qt * 512, (qt + 1) * 512)
            s_ps = sp.tile([128, 1024], F32, tag="s")
            for j in range(2):
                kb = 2 * g + j
                mm(s_ps[:, j * 512:(j + 1) * 512],
                   kT(h)[:, kb * 128:(kb + 1) * 128], qT(h)[:, qs],
                   True, True)
            px = pxp.tile([128, 1024], BF16, tag="px", bufs=28)
            nc.scalar.activation(px[:], s_ps[:], AF.Exp, bias=0.0, scale=0.125)
            px_tiles[(p, g)] = px

        def close_gen(p):
            if p == 0:
                ensure_done(vg)   # PV reads v3; emission-order guard
            h, qt = phases[p]
            o_ps = op.tile([128, 4, 65], F32, tag="o", bufs=1)
            for qb in range(4):
                for g in range(NG):
                    px = px_tiles[(p, g)]
                    for j in range(2):
                        kb = 2 * g + j
                        mm(o_ps[:, qb, :],
                           px[:, j * 512 + qb * 128:j * 512 + (qb + 1) * 128],
                           v3[:, kb, h, :],
                           qb == 0 and kb == 0, kb == KB - 1)
                if qb == 1 or qb == 3:
                    yield
            # normalize by the ones-column denominators (batched reciprocal,
            # then per-qb per-partition multiply); epilogues after ALL PV so
            # coarse WAR tracking can't serialize the qb bundles
            rec4 = pipe.tile([128, 4], F32, tag="rec4", name="rec4")
            nc.vector.reciprocal(rec4[:], o_ps[:, :, 64])
            for qb in range(4):
                dst = (o2[:, qt, qb, h * 64:(h + 1) * 64] if h < 2
                       else o1[:, qt, qb, :])
                nc.vector.tensor_scalar(dst, o_ps[:, qb, 0:64],
                                        rec4[:, qb:qb + 1], None, ALU.mult)
                if h == 2:
                    proj_qb(qt, qb)
                yield
            for g in range(NG):
                del px_tiles[(p, g)]

        def drain_close(cg):
            for _ in cg:
                pass

        # ---------- main schedule ----------
        # Phase 0 runs with qkv(h0) inlined per tile: tile t unlocks S groups
        # 2t, 2t+1 (k-tiles) while qT(qt0) comes entirely from tile 0.
        qg0 = qkv_gen(0)
        vg = v_gen()
        g1, g2 = qkv_gen(1), qkv_gen(2)
        fillers.append(vg)
        fillers.append(g1)
        need_gen = {1: g1, 2: g2}

        def drain_n(gen, n_chunks):
            for _ in range(n_chunks):
                next(gen)

        # Front: dense qkv-h0 mms with the exp stream fed by EVERY group
        # whose gates are open. Tile t gates k-blocks 4t..4t+3 (groups
        # 2t,2t+1 of every h0 phase) and the q-tokens of phase (0,t).
        drain_n(qg0, 5)              # thru finB(0)
        front = [(0, 0), (0, 1),
                 "T1", (0, 2), (0, 3), (1, 0), (1, 1),
                 "T2", (0, 4), (0, 5), (1, 2), (1, 3), (2, 0), (2, 1),
                 "T3", (0, 6), (0, 7), (1, 4), (1, 5), (2, 2), (2, 3),
                 (3, 0), (3, 1)]
        for item in front:
            if item == "T1":
                drain_n(qg0, 3)      # finA2, mms3, finB1
            elif item == "T2":
                drain_n(qg0, 2)      # finA3, finB2
            elif item == "T3":
                drain_n(qg0, 1)      # finB3
            else:
                emit_group(*item)
                pump(600)

        # Steady state: early closes deferred ~2 phases (px ring holds ~3
        # phases) so v/qkv fillers use the early PE slack; late closes pulled
        # in so proj work overlaps the remaining exp stream.
        NP = len(phases)
        close_at = {}
        for p in range(NP - 1):
            # early closes deferred 3 phases (shifts PV out of the PE-heavy
            # qkv/v window); h2 closes pulled in so proj overlaps exps
            lag = (p + 2, NG - 1) if p < 8 else (p, NG - 1)
            close_at.setdefault(lag, []).append(p)

        def after_close(cp):
            if phases[cp] == (0, 2):
                fillers.append(g2)

        active_closes = []
        for p in range(1, NP):
            nh = phases[p][0]
            if nh != phases[p - 1][0]:
                ensure_done(need_gen[nh])
            for g in range(NG):
                emit_group(p, g)
                pump(2400)
                for cp in close_at.get((p, g), []):
                    active_closes.append((cp, close_gen(cp)))
                if active_closes:
                    steps = 2 if p >= NP - 2 else 1
                    for _ in range(steps):
                        if not active_closes:
                            break
                        cp, cg = active_closes[0]
                        try:
                            next(cg)
                        except StopIteration:
                            active_closes.pop(0)
                            after_close(cp)
        for cp, cg in active_closes:
            drain_close(cg)
            after_close(cp)
        drain_close(close_gen(NP - 1))
        while fillers:
            pump(100000)

    if split_waits:
        _split_waits(nc)
    return nc


def _split_waits(nc):
    """Walrus lowers at most one sync-wait per instruction; move excess waits
    onto NoOps inserted just before, on the same engine queue."""
    k = 0
    for fn in nc.m.functions:
        for bb in fn.blocks:
            il = bb.instructions
            idx = 0
            while idx < len(il):
                inst = il[idx]
                si = inst.sync_info
                eng = getattr(inst, "engine", None)
                if (si is not None and len(si.on_wait) > 1
                        and eng is not None
                        and str(eng) != "EngineType.Unassigned"):
                    waits = list(si.on_wait)
                    inst.sync_info = mybir.SyncInfo(
                        on_wait=[waits[-1]], on_update=list(si.on_update))
                    for w in waits[:-1]:
                        nop = mybir.InstNoOp(
                            name=f"I-waitnop-{k}", engine=eng, ins=[], outs=[],
                            sync_info=mybir.SyncInfo(on_wait=[w], on_update=[]))
                        k += 1
                        il.insert(idx, nop)
                        idx += 1
                idx += 1


def _prep_core_inputs(core, x, rope_cos, rope_sin, qkv_kernel, qkv_bias,
                      proj_kernel, proj_bias, q_norm_w, k_norm_w):
    b = core // 4
    heads = [3 * (core % 4) + i for i in range(HP)]

    wq = qkv_kernel.reshape(C, 3, H, HD)
    bq = qkv_bias.reshape(3, H, HD)

    xTa = np.ascontiguousarray(x[b].T).astype(BF)

    wqk = np.empty((C, HP * 128), np.float32)
    bqk = np.zeros((128, HP), np.float32)
    for i, h in enumerate(heads):
        wqk[:, i * 128:i * 128 + 64] = wq[:, 0, h, PERM]
        wqk[:, i * 128 + 64:(i + 1) * 128] = wq[:, 1, h, PERM]
        bqk[0:64, i] = bq[0, h, PERM]
        bqk[64:128, i] = bq[1, h, PERM]

    wv = np.zeros((C, HP * 64), np.float32)
    for i, h in enumerate(heads):
        wv[:, i * 64:(i + 1) * 64] = wq[:, 2, h, :]
    # packed [p, (c m)] so the SBUF copy is one dense DMA
    wvp = wv.reshape(CCH, 128, HP * 64).transpose(1, 0, 2).reshape(128, -1)

    cosT = rope_cos.T  # (HD, N)
    sinT = rope_sin.T
    cosw = np.empty((128, N), np.float32)
    sinw = np.empty((128, N), np.float32)
    cosw[0:64] = cosT[PERM] * q_norm_w[PERM][:, None]
    cosw[64:128] = cosT[PERM] * k_norm_w[PERM][:, None]
    # sin multiplies the SHUFFLED (partner) value -> partner's norm weight
    qn_p = q_norm_w[PERM][SWAPIDX]
    kn_p = k_norm_w[PERM][SWAPIDX]
    sinw[0:64] = SIGN[:, None] * sinT[PERM] * qn_p[:, None]
    sinw[64:128] = SIGN[:, None] * sinT[PERM] * kn_p[:, None]

    onesp = np.zeros((128, 2), np.float32)
    onesp[0:64, 0] = 1.0
    onesp[64:128, 1] = 1.0

    sel4 = np.zeros((128, 512), np.float32)
    for t in range(NT):
        sel4[32 * t, t * 128:t * 128 + 64] = 1.0
        sel4[32 * t + 1, t * 128 + 64:(t + 1) * 128] = 1.0

    rows01 = np.concatenate([np.arange(h * HD, (h + 1) * HD)
                             for h in heads[0:2]])
    rows2 = np.arange(heads[2] * HD, (heads[2] + 1) * HD)
    wp01 = proj_kernel[rows01, :]
    wp2 = proj_kernel[rows2, :]

    consts = np.zeros((128, 642), np.float32)
    consts[:, 0:2] = onesp
    consts[:, 2:514] = sel4
    consts[:, 514:642] = np.eye(128, dtype=np.float32)
    return {"xT": xTa, "wqk": wqk.astype(BF), "bqk": bqk,
            "cosw": cosw.astype(BF), "sinw": sinw.astype(BF),
            "wvp": np.ascontiguousarray(wvp).astype(BF),
            "wp01": np.ascontiguousarray(wp01).astype(BF),
            "wp2": np.ascontiguousarray(wp2).astype(BF),
            "consts": consts.astype(BF)}


def kernel(x, rope_cos, rope_sin, qkv_kernel, qkv_bias, proj_kernel,
           proj_bias, q_norm_w, k_norm_w, _trace=False):
    args = [np.asarray(a, dtype=np.float32) for a in
            (x, rope_cos, rope_sin, qkv_kernel, qkv_bias, proj_kernel,
             proj_bias, q_norm_w, k_norm_w)]
    in_maps = [_prep_core_inputs(c, *args) for c in range(NCORES)]

    if "nc" not in _NC_CACHE:
        _NC_CACHE["nc"] = build_nc()
    nc = _NC_CACHE["nc"]

    res = run_bass_kernel_spmd(nc, in_maps, core_ids=list(range(NCORES)),
                               trace=_trace)
    parts = [np.asarray(res.results[c]["out"]).astype(np.float32)
             for c in range(NCORES)]
    # v-bias contributes exactly bv @ proj_kernel (softmax rows sum to 1)
    pb = (np.asarray(proj_bias, dtype=np.float32)
          + np.asarray(qkv_bias, dtype=np.float32)[2 * C:]
          @ np.asarray(proj_kernel, dtype=np.float32))
    out = np.empty((B, N, C), np.float32)
    for b in range(B):
        out[b] = (parts[4 * b] + parts[4 * b + 1] + parts[4 * b + 2]
                  + parts[4 * b + 3] + pb)
    if _trace:
        kernel.last_results = res
    return out


# revision 12
# speedup vs baseline: 1.0108x; 1.0028x over previous
"""Multi-head attention (RMSNorm-QK + RoPE + softmax + proj) on 8 Trainium2 cores.

v2 design (cost-model-driven rewrite of the baseline):
 - bf16 operands everywhere (matmuls cost 1 cyc/row like fp32r, but DVE gets
   2x modes and DMA halves); fp32 PSUM accumulation throughout.
 - Transposed PV: O tiles are [128 q, 65] (64 dims + ones col for the softmax
   denominator), using all 128 output partitions -> PV drops from 32768 to
   16640 cyc/head, the denominator becomes a per-partition column (reciprocal
   + tensor_scalar mult), and the old broadcast-reciprocal matmuls vanish.
 - O^T for the projection comes from PE transposes (128 bf16 rows each).
 - RMS rsqrt via exp(-0.5 ln x) on ACT (same table as the softmax exp; the
   DVE has no pow/divide/rsqrt ISA), qkv bias added in the DVE pipeline
   (per-partition scalar), v bias folded into the host-side proj bias
   (softmax rows sum to 1), softmax denominators via batched DVE reciprocal.
 - RoPE elementwise work split DVE/Pool; emission order software-pipelines
   S(k+1) ahead of exp(k), stages a phase's px tiles in SBUF so each O
   qb-region accumulates contiguously (PSUM start bit stays per-element
   correct on HW), defers phase closes ~1.25 phases so early PE work (qkv+v)
   overlaps the ACT-bound exp stream, and pumps qkv/v/proj filler chunks into
   the PE gaps.

Sharding: core c handles batch c//4 and heads [3*(c%4), 3*(c%4)+3).
Each core writes a bf16 [N, C] partial; the host sums 4 partials per batch
and adds proj_bias + qkv_bias[v-part] @ proj_kernel.
"""
import sys

for _p in ("/opt/trn_rl_repo", "/opt/trn_rl_repo/concourse"):
    if _p not in sys.path:
        sys.path.insert(0, _p)

from collections import deque
from contextlib import ExitStack

import ml_dtypes
import numpy as np

import concourse.bass as bass
import concourse.mybir as mybir
import concourse.tile as tile
from concourse.bass_utils import run_bass_kernel_spmd

F32 = mybir.dt.float32
BF16 = mybir.dt.bfloat16
AF = mybir.ActivationFunctionType
ALU = mybir.AluOpType
BF = ml_dtypes.bfloat16

B, N, C = 2, 2048, 768
H, HD = 12, 64
HP = 3            # heads per core
NCORES = 8
CCH = 6           # contraction chunks of 128
NT = 4            # token tiles of 512
KB = 16           # k blocks of 128
NG = 8            # 2-kb groups per (head, qtile) phase

SWAP_MASK = [(i + 16) % 32 for i in range(32)]
PERM = np.concatenate([np.arange(0, 16), np.arange(32, 48),
                       np.arange(16, 32), np.arange(48, 64)])
SIGN = np.where(PERM < 32, -1.0, 1.0).astype(np.float32)
# rope partner of PERM-position p (SWAP_MASK's intra-32 half swap)
SWAPIDX = np.array([(p // 32) * 32 + (p + 16) % 32 for p in range(64)])

_NC_CACHE = {}


def build_nc(split_waits=True):
    nc = bass.Bass(target_bir_lowering=True)
    xT = nc.declare_dram_parameter("xT", [C, N], BF16, isOutput=False)
    wqk = nc.declare_dram_parameter("wqk", [C, HP * 128], BF16, isOutput=False)
    cosw = nc.declare_dram_parameter("cosw", [128, N], BF16, isOutput=False)
    sinw = nc.declare_dram_parameter("sinw", [128, N], BF16, isOutput=False)
    wvp = nc.declare_dram_parameter("wvp", [128, CCH * HP * 64], BF16,
                                    isOutput=False)
    wp01 = nc.declare_dram_parameter("wp01", [128, C], BF16, isOutput=False)
    wp2 = nc.declare_dram_parameter("wp2", [64, C], BF16, isOutput=False)
    # consts: [onesp(2) | sel4(512) | ident(128)]
    consts = nc.declare_dram_parameter("consts", [128, 642], BF16,
                                       isOutput=False)
    bqk = nc.declare_dram_parameter("bqk", [128, HP], F32, isOutput=False)
    out = nc.declare_dram_parameter("out", [N, C], BF16, isOutput=True)

    with tile.TileContext(nc) as tc, ExitStack() as ctx:
        sb = ctx.enter_context(tc.tile_pool(name="sb", bufs=1))
        pipe = ctx.enter_context(tc.tile_pool(name="pipe", bufs=2))
        pxp = pipe     # per-tag bufs below
        otp = pipe
        pop = pipe
        # PSUM: 4 + 2 + 1 + 1 = 8 banks, one pool with per-tag bufs
        sp = ctx.enter_context(tc.tile_pool(name="sp", bufs=2, space="PSUM"))
        qp = sp
        op = sp
        mp = sp

        # ---------- static SBUF tiles ----------
        xs = sb.tile([128, CCH, N], BF16, tag="xs")
        wqk_sb = sb.tile([128, CCH, HP * 128], BF16, tag="wqk")
        wv_sb = sb.tile([128, CCH, HP * 64], BF16, tag="wv")
        cos_sb = sb.tile([128, N], BF16, tag="cos")
        sin_sb = sb.tile([128, N], BF16, tag="sin")
        cn = sb.tile([128, 642], BF16, tag="cn")
        onesp_sb = cn[:, 0:2]
        sel_sb = cn[:, 2:514]
        ident_sb = cn[:, 514:642]
        bqk_sb = sb.tile([128, HP], F32, tag="bqk")
        wp01_sb = sb.tile([128, C], BF16, tag="wp01")
        wp2_sb = sb.tile([64, C], BF16, tag="wp2")

        q12 = sb.tile([128, N], BF16, tag="q12")
        k12 = sb.tile([128, N], BF16, tag="k12")
        q3 = sb.tile([64, N], BF16, tag="q3")
        k3 = sb.tile([64, N], BF16, tag="k3")
        t4_all = sb.tile([128, N], BF16, tag="t4_all")
        s_sb = sb.tile([128, 512], F32, tag="s_sb")
        sv = sb.tile([128, 512], BF16, tag="sv")
        v3 = sb.tile([128, KB, HP, 65], BF16, tag="v3")
        ones48 = sb.tile([128, KB * HP], BF16, tag="ones48")
        o2 = sb.tile([128, NT, 4, 128], BF16, tag="o2")
        o1 = sb.tile([128, NT, 4, 64], BF16, tag="o1")

        def qT(h):
            return (q12[0:64], q12[64:128], q3[:])[h]

        def kT(h):
            return (k12[0:64], k12[64:128], k3[:])[h]

        # ---------- prologue DMAs (ordered for earliest qkv start) ----------
        xT_r = xT[:].rearrange("(c p) n -> p c n", p=128)
        wqk_r = wqk[:].rearrange("(c p) m -> p c m", p=128)
        d = nc.sync.dma_start
        d(wqk_sb[:, 0:2, :], wqk_r[:, 0:2, :])
        d(xs[:, 0:3, 0:512], xT_r[:, 0:3, 0:512])      # tile-0 tokens
        d(wqk_sb[:, 2:6, :], wqk_r[:, 2:6, :])
        d(xs[:, 3:6, 0:512], xT_r[:, 3:6, 0:512])
        d(bqk_sb[:], bqk[:, :])
        d(cn[:], consts[:, :])
        d(cos_sb[:, 0:1024], cosw[:, 0:1024])
        d(sin_sb[:, 0:1024], sinw[:, 0:1024])
        d(xs[:, :, 512:1024], xT_r[:, :, 512:1024])
        d(xs[:, :, 1024:1536], xT_r[:, :, 1024:1536])
        d(wv_sb[:].rearrange("p c m -> p (c m)"), wvp[:, :])
        d(xs[:, :, 1536:2048], xT_r[:, :, 1536:2048])
        d(cos_sb[:, 1024:2048], cosw[:, 1024:2048])
        d(sin_sb[:, 1024:2048], sinw[:, 1024:2048])
        d(wp01_sb[:], wp01[:, :])
        d(wp2_sb[:], wp2[:, :])

        nc.vector.memset(sv[:], 1.0)   # rows never written stay 1 (sel zeros them)
        nc.vector.memset(s_sb[:], 1.0)
        nc.vector.memset(ones48[:], 1.0)
        nc.vector.tensor_copy(
            v3[:].rearrange("p a b n -> p (a b) n", n=65)[:, :, 64], ones48[:])

        def mm(out_ap, lhsT, rhs, start, stop):
            nc.tensor.matmul(out_ap, lhsT, rhs, start=start, stop=stop,
                             skip_group_check=True)

        # ---------- qkv-head generator ----------
        # Per-tile chunks: mms -> RoPE pipe -> finA (sumsq+rsqrt) ->
        # finB (broadcast+scale). Emission defers fins so PE stays dense;
        # the qp ring (2) tolerates exactly one deferred finA.
        def qkv_gen(h):
            hs = slice(h * 128, (h + 1) * 128)
            qk = [None] * NT

            def mms(t):
                ts = slice(t * 512, (t + 1) * 512)
                qk[t] = qp.tile([128, 512], F32, tag="q", name=f"qk{t}")
                for c in range(CCH):
                    mm(qk[t][:], wqk_sb[:, c, hs], xs[:, c, ts], c == 0,
                       c == CCH - 1)

            def rope(t):
                ts = slice(t * 512, (t + 1) * 512)
                qkb = pipe.tile([128, 512], BF16, tag="qkb")
                nc.vector.tensor_scalar(qkb[:], qk[t][:], bqk_sb[:, h:h + 1],
                                        None, ALU.add)
                sq = pipe.tile([128, 512], BF16, tag="sq")
                if h == 0:
                    nc.vector.tensor_mul(sq[:], qkb[:], qkb[:])
                t1 = pipe.tile([128, 512], BF16, tag="t1")
                nc.gpsimd.tensor_mul(t1[:], qkb[:], cos_sb[:, ts])
                t2 = pipe.tile([128, 512], BF16, tag="t2")
                nc.vector.stream_shuffle(t2[:], qkb[:], SWAP_MASK)
                t3 = pipe.tile([128, 512], BF16, tag="t3")
                nc.vector.tensor_mul(t3[:], t2[:], sin_sb[:, ts])
                nc.vector.tensor_add(t4_all[:, ts], t1[:], t3[:])
                if h != 0:
                    nc.gpsimd.tensor_mul(sq[:], qkb[:], qkb[:])
                return sq

            def finA(t, sq):
                rows = slice(32 * t, 32 * t + 2)
                mm(qk[t][0:2, :], onesp_sb[:], sq[:], True, True)
                if h == 0:
                    # rsqrt = exp(-0.5 ln(ms)); same ACT table as softmax exp
                    lv = pipe.tile([2, 512], F32, tag="lv", name="lv")
                    nc.scalar.activation(lv[:], qk[t][0:2, :], AF.Ln,
                                         bias=0.0, scale=1.0 / HD)
                    nc.scalar.activation(sv[rows, :], lv[:], AF.Exp,
                                         bias=0.0, scale=-0.5)
                else:
                    nc.vector.tensor_copy(s_sb[rows, :], qk[t][0:2, :])

            def lnexp():
                lva = pipe.tile([128, 512], F32, tag="lva", name="lva")
                nc.scalar.activation(lva[:], s_sb[:], AF.Ln,
                                     bias=0.0, scale=1.0 / HD)
                nc.scalar.activation(sv[:], lva[:], AF.Exp, bias=0.0,
                                     scale=-0.5)

            def finB(t):
                ts = slice(t * 512, (t + 1) * 512)
                sqk_ps = qp.tile([128, 512], F32, tag="q")
                mm(sqk_ps[:], sel_sb[:, t * 128:(t + 1) * 128], sv[:],
                   True, True)
                nc.vector.tensor_mul(qT(h)[:, ts], t4_all[0:64, ts],
                                     sqk_ps[0:64, :])
                nc.vector.tensor_mul(kT(h)[:, ts], t4_all[64:128, ts],
                                     sqk_ps[64:128, :])

            sqs = [None] * NT

            def do_mms(t):
                mms(t)
                sqs[t] = rope(t)

            do_mms(0)
            yield 4500
            do_mms(1)
            yield 4500
            finA(0, sqs[0])
            yield 700
            do_mms(2)
            yield 4500
            finA(1, sqs[1])
            if h == 0:
                finB(0)
            yield 1600
            finA(2, sqs[2])
            yield 700
            do_mms(3)
            yield 4500
            if h == 0:
                finB(1)
            yield 900
            finA(3, sqs[3])
            yield 700
            if h != 0:
                lnexp()
                yield 700
                finB(0)
                yield 900
                finB(1)
                yield 900
            finB(2)
            yield 900
            finB(3)
            yield 900

        # ---------- v generator ----------
        def v_gen():
            for tt in range(KB):
                v_ps = qp.tile([128, HP * 64], F32, tag="q")
                for c in range(CCH):
                    mm(v_ps[:], xs[:, c, tt * 128:(tt + 1) * 128],
                       wv_sb[:, c, :], c == 0, c == CCH - 1)
                nc.vector.tensor_copy(
                    v3[:, tt, :, 0:64],
                    v_ps[:, :].rearrange("p (h n) -> p h n", h=HP))
                yield 1500

        # ---------- proj of one (qtile, qblock) ----------
        mtr = [None]

        def proj_qb(qt, qb):
            if mtr[0] is None:
                mtr[0] = mp.tile([128, 4, 128], F32, tag="m", name="mtr", bufs=1)
            m = mtr[0]
            tr01 = m[:, qb, 0:64].bitcast(BF16)
            tr2 = m[0:64, qb, 64:128].bitcast(BF16)
            nc.tensor.transpose(tr01, o2[:, qt, qb, :], ident_sb[:])
            nc.tensor.transpose(tr2, o1[:, qt, qb, :], ident_sb[:])
            on_act = qt == 3   # ACT is idle once the last exps drain
            ot01 = otp.tile([128, 128], BF16, tag="ot01", bufs=6)
            ot2 = otp.tile([64, 128], BF16, tag="ot2", bufs=6)
            if on_act:
                nc.scalar.activation(ot01[:], tr01, AF.Copy, bias=0.0,
                                     scale=1.0)
                nc.vector.tensor_copy(ot2[:], tr2)
            else:
                nc.vector.tensor_copy(ot01[:], tr01)
                nc.vector.tensor_copy(ot2[:], tr2)
            po = pop.tile([128, C], BF16, tag="po", bufs=3)
            for half in range(2):
                cs = slice(half * 384, (half + 1) * 384)
                p_ps = qp.tile([128, 384], F32, tag="q")
                mm(p_ps[:], ot01[:], wp01_sb[:, cs], True, False)
                mm(p_ps[:], ot2[:], wp2_sb[:, cs], False, True)
                if on_act and half == 1:
                    nc.scalar.activation(po[:, cs], p_ps[:], AF.Copy,
                                         bias=0.0, scale=1.0)
                else:
                    nc.vector.tensor_copy(po[:, cs], p_ps[:])
            tb = qt * 4 + qb
            nc.sync.dma_start(out[tb * 128:(tb + 1) * 128, :], po[:])

        # ---------- filler pump ----------
        fillers = deque()
        debt = [0.0]

        def pump(budget):
            budget += debt[0]
            while budget > 0 and fillers:
                try:
                    budget -= next(fillers[0])
                except StopIteration:
                    fillers.popleft()
            debt[0] = min(budget, 3000.0)

        def ensure_done(gen):
            """Pump until `gen` has fully emitted (emission-order guard for
            cross-generator data deps)."""
            while gen in fillers:
                pump(100000)

        # ---------- attention stream ----------
        phases = [(h, qt) for h in range(HP) for qt in range(NT)]
        px_tiles = {}
        emitted = set()

        def emit_group(p, g):
            if (p, g) in emitted:
                return
            emitted.add((p, g))
            h, qt = phases[p]
            qs = slice(qt * 512, (qt + 1) * 512)
            s_ps = sp.tile([128, 1024], F32, tag="s")
            for j in range(2):
                kb = 2 * g + j
                mm(s_ps[:, j * 512:(j + 1) * 512],
                   kT(h)[:, kb * 128:(kb + 1) * 128], qT(h)[:, qs],
                   True, True)
            px = pxp.tile([128, 1024], BF16, tag="px", bufs=28)
            nc.scalar.activation(px[:], s_ps[:], AF.Exp, bias=0.0, scale=0.125)
            px_tiles[(p, g)] = px

        def close_gen(p):
            if p == 0:
                ensure_done(vg)   # PV reads v3; emission-order guard
            h, qt = phases[p]
            o_ps = op.tile([128, 4, 65], F32, tag="o", bufs=1)
            for qb in range(4):
                for g in range(NG):
                    px = px_tiles[(p, g)]
                    for j in range(2):
                        kb = 2 * g + j
                        mm(o_ps[:, qb, :],
                           px[:, j * 512 + qb * 128:j * 512 + (qb + 1) * 128],
                           v3[:, kb, h, :],
                           qb == 0 and kb == 0, kb == KB - 1)
                if qb == 1 or qb == 3:
                    yield
            # normalize by the ones-column denominators (batched reciprocal,
            # then per-qb per-partition multiply); epilogues after ALL PV so
            # coarse WAR tracking can't serialize the qb bundles
            rec4 = pipe.tile([128, 4], F32, tag="rec4", name="rec4")
            nc.vector.reciprocal(rec4[:], o_ps[:, :, 64])
            for qb in range(4):
                dst = (o2[:, qt, qb, h * 64:(h + 1) * 64] if h < 2
                       else o1[:, qt, qb, :])
                nc.vector.tensor_scalar(dst, o_ps[:, qb, 0:64],
                                        rec4[:, qb:qb + 1], None, ALU.mult)
                if h == 2:
                    proj_qb(qt, qb)
                yield
            for g in range(NG):
                del px_tiles[(p, g)]

        def drain_close(cg):
            for _ in cg:
                pass

        # ---------- main schedule ----------
        # Phase 0 runs with qkv(h0) inlined per tile: tile t unlocks S groups
        # 2t, 2t+1 (k-tiles) while qT(qt0) comes entirely from tile 0.
        qg0 = qkv_gen(0)
        vg = v_gen()
        g1, g2 = qkv_gen(1), qkv_gen(2)
        fillers.append(vg)
        fillers.append(g1)
        need_gen = {1: g1, 2: g2}

        def drain_n(gen, n_chunks):
            for _ in range(n_chunks):
                next(gen)

        # Front: dense qkv-h0 mms with the exp stream fed by EVERY group
        # whose gates are open. Tile t gates k-blocks 4t..4t+3 (groups
        # 2t,2t+1 of every h0 phase) and the q-tokens of phase (0,t).
        drain_n(qg0, 5)              # thru finB(0)
        front = [(0, 0), (0, 1),
                 "T1", (1, 0), (0, 2), (1, 1), (0, 3),
                 "T2", (2, 0), (1, 2), (0, 4), (2, 1), (1, 3), (0, 5),
                 "T3", (3, 0), (2, 2), (1, 4), (0, 6), (3, 1), (2, 3),
                 (1, 5), (0, 7)]
        for item in front:
            if item == "T1":
                drain_n(qg0, 3)      # finA2, mms3, finB1
            elif item == "T2":
                drain_n(qg0, 2)      # finA3, finB2
            elif item == "T3":
                drain_n(qg0, 1)      # finB3
            else:
                emit_group(*item)
                pump(600)

        # Steady state: early closes deferred ~2 phases (px ring holds ~3
        # phases) so v/qkv fillers use the early PE slack; late closes pulled
        # in so proj work overlaps the remaining exp stream.
        NP = len(phases)
        close_at = {}
        for p in range(NP - 1):
            # early closes deferred 3 phases (shifts PV out of the PE-heavy
            # qkv/v window); h2 closes pulled in so proj overlaps exps
            lag = (p + 2, NG - 1) if p < 8 else (p, NG - 1)
            close_at.setdefault(lag, []).append(p)

        def after_close(cp):
            if phases[cp] == (0, 2):
                fillers.append(g2)

        active_closes = []
        for p in range(1, NP):
            nh = phases[p][0]
            if nh != phases[p - 1][0]:
                ensure_done(need_gen[nh])
            for g in range(NG):
                emit_group(p, g)
                pump(2400)
                for cp in close_at.get((p, g), []):
                    active_closes.append((cp, close_gen(cp)))
                if active_closes:
                    steps = 2 if p >= NP - 2 else 1
                    for _ in range(steps):
                        if not active_closes:
                            break
                        cp, cg = active_closes[0]
                        try:
                            next(cg)
                        except StopIteration:
                            active_closes.pop(0)
                            after_close(cp)
        for cp, cg in active_closes:
            drain_close(cg)
            after_close(cp)
        drain_close(close_gen(NP - 1))
        while fillers:
            pump(100000)

    if split_waits:
        _split_waits(nc)
    return nc


def _split_waits(nc):
    """Walrus lowers at most one sync-wait per instruction; move excess waits
    onto NoOps inserted just before, on the same engine queue."""
    k = 0
    for fn in nc.m.functions:
        for bb in fn.blocks:
            il = bb.instructions
            idx = 0
            while idx < len(il):
                inst = il[idx]
                si = inst.sync_info
                eng = getattr(inst, "engine", None)
                if (si is not None and len(si.on_wait) > 1
                        and eng is not None
                        and str(eng) != "EngineType.Unassigned"):
                    waits = list(si.on_wait)
                    inst.sync_info = mybir.SyncInfo(
                        on_wait=[waits[-1]], on_update=list(si.on_update))
                    for w in waits[:-1]:
                        nop = mybir.InstNoOp(
                            name=f"I-waitnop-{k}", engine=eng, ins=[], outs=[],
                            sync_info=mybir.SyncInfo(on_wait=[w], on_update=[]))
                        k += 1
                        il.insert(idx, nop)
                        idx += 1
                idx += 1


def _prep_core_inputs(core, x, rope_cos, rope_sin, qkv_kernel, qkv_bias,
                      proj_kernel, proj_bias, q_norm_w, k_norm_w):
    b = core // 4
    heads = [3 * (core % 4) + i for i in range(HP)]

    wq = qkv_kernel.reshape(C, 3, H, HD)
    bq = qkv_bias.reshape(3, H, HD)

    xTa = np.ascontiguousarray(x[b].T).astype(BF)

    wqk = np.empty((C, HP * 128), np.float32)
    bqk = np.zeros((128, HP), np.float32)
    for i, h in enumerate(heads):
        wqk[:, i * 128:i * 128 + 64] = wq[:, 0, h, PERM]
        wqk[:, i * 128 + 64:(i + 1) * 128] = wq[:, 1, h, PERM]
        bqk[0:64, i] = bq[0, h, PERM]
        bqk[64:128, i] = bq[1, h, PERM]

    wv = np.zeros((C, HP * 64), np.float32)
    for i, h in enumerate(heads):
        wv[:, i * 64:(i + 1) * 64] = wq[:, 2, h, :]
    # packed [p, (c m)] so the SBUF copy is one dense DMA
    wvp = wv.reshape(CCH, 128, HP * 64).transpose(1, 0, 2).reshape(128, -1)

    cosT = rope_cos.T  # (HD, N)
    sinT = rope_sin.T
    cosw = np.empty((128, N), np.float32)
    sinw = np.empty((128, N), np.float32)
    cosw[0:64] = cosT[PERM] * q_norm_w[PERM][:, None]
    cosw[64:128] = cosT[PERM] * k_norm_w[PERM][:, None]
    # sin multiplies the SHUFFLED (partner) value -> partner's norm weight
    qn_p = q_norm_w[PERM][SWAPIDX]
    kn_p = k_norm_w[PERM][SWAPIDX]
    sinw[0:64] = SIGN[:, None] * sinT[PERM] * qn_p[:, None]
    sinw[64:128] = SIGN[:, None] * sinT[PERM] * kn_p[:, None]

    onesp = np.zeros((128, 2), np.float32)
    onesp[0:64, 0] = 1.0
    onesp[64:128, 1] = 1.0

    sel4 = np.zeros((128, 512), np.float32)
    for t in range(NT):
        sel4[32 * t, t * 128:t * 128 + 64] = 1.0
        sel4[32 * t + 1, t * 128 + 64:(t + 1) * 128] = 1.0

    rows01 = np.concatenate([np.arange(h * HD, (h + 1) * HD)
                             for h in heads[0:2]])
    rows2 = np.arange(heads[2] * HD, (heads[2] + 1) * HD)
    wp01 = proj_kernel[rows01, :]
    wp2 = proj_kernel[rows2, :]

    consts = np.zeros((128, 642), np.float32)
    consts[:, 0:2] = onesp
    consts[:, 2:514] = sel4
    consts[:, 514:642] = np.eye(128, dtype=np.float32)
    return {"xT": xTa, "wqk": wqk.astype(BF), "bqk": bqk,
            "cosw": cosw.astype(BF), "sinw": sinw.astype(BF),
            "wvp": np.ascontiguousarray(wvp).astype(BF),
            "wp01": np.ascontiguousarray(wp01).astype(BF),
            "wp2": np.ascontiguousarray(wp2).astype(BF),
            "consts": consts.astype(BF)}


def kernel(x, rope_cos, rope_sin, qkv_kernel, qkv_bias, proj_kernel,
           proj_bias, q_norm_w, k_norm_w, _trace=False):
    args = [np.asarray(a, dtype=np.float32) for a in
            (x, rope_cos, rope_sin, qkv_kernel, qkv_bias, proj_kernel,
             proj_bias, q_norm_w, k_norm_w)]
    in_maps = [_prep_core_inputs(c, *args) for c in range(NCORES)]

    if "nc" not in _NC_CACHE:
        _NC_CACHE["nc"] = build_nc()
    nc = _NC_CACHE["nc"]

    res = run_bass_kernel_spmd(nc, in_maps, core_ids=list(range(NCORES)),
                               trace=_trace)
    parts = [np.asarray(res.results[c]["out"]).astype(np.float32)
             for c in range(NCORES)]
    # v-bias contributes exactly bv @ proj_kernel (softmax rows sum to 1)
    pb = (np.asarray(proj_bias, dtype=np.float32)
          + np.asarray(qkv_bias, dtype=np.float32)[2 * C:]
          @ np.asarray(proj_kernel, dtype=np.float32))
    out = np.empty((B, N, C), np.float32)
    for b in range(B):
        out[b] = (parts[4 * b] + parts[4 * b + 1] + parts[4 * b + 2]
                  + parts[4 * b + 3] + pb)
    if _trace:
        kernel.last_results = res
    return out
